# revision 1
# baseline (speedup 1.0000x reference)
"""DeepseekV3 MLA attention prefill (S=1024, H=128 heads, HID=7168) on 8 TRN2
NeuronCores.

Sharding: tensor-parallel over heads (16 heads/core) for q_b/kv_b/attention/
o_proj; the low-rank input projections (q_a / kv_a) are sequence-sharded
(128 rows/core) and exchanged with one small AllGather of the transposed,
rms-normed activations. Each core emits a partial output projection
(contraction over its own 16 heads); the host sums the 8 partials.

All matmuls run as float32r (full-rate fp32 mode on the PE); softmax and
normalization math stays float32.
"""
import math
import numpy as np

import concourse.bass as bass
import concourse.mybir as mybir
import concourse.bacc as bacc
import concourse.tile as tile
import concourse.bass_utils as bass_utils
from concourse.masks import make_identity
from contextlib import ExitStack

F32 = mybir.dt.float32
F32R = mybir.dt.float32r
AF = mybir.ActivationFunctionType
OP = mybir.AluOpType

N_CORES = 8
S = 1024
HID = 7168
H = 128
HG = H // N_CORES          # 16 heads per core
D_NOPE = 128
D_ROPE = 64
D_Q = D_NOPE + D_ROPE      # 192
D_V = 128
CQ = 1536                  # q lora rank
CKV = 512                  # kv lora rank
CA = CQ + CKV + D_ROPE     # 2112 fused a-proj cols
S_SH = S // N_CORES        # 128 sequence rows per core
CC_A = HID // 128          # 56 contraction chunks for a-proj
NT_A = [(0, 512), (512, 512), (1024, 512), (1536, 512), (2048, 64)]
SCALE = 1.0 / math.sqrt(D_Q)
EPS = 1e-6
G_HEADS = 2                # heads per group
N_GROUPS = HG // G_HEADS   # 8 groups
QT = 2                     # q-tiles of 512 per head
LAST_EXEC_NS = None

_CACHE = {}


def _build_nc():
    nc = bacc.Bacc("TRN2", target_bir_lowering=False, debug=False,
                   num_devices=N_CORES)

    xT = nc.dram_tensor("xT", [HID, S_SH], F32R, kind="ExternalInput")
    wa = nc.dram_tensor("wa", [HID, CA], F32R, kind="ExternalInput")
    qbn = nc.dram_tensor("qbn", [CQ, HG * D_NOPE], F32R, kind="ExternalInput")
    qbp = nc.dram_tensor("qbp", [CQ, HG * D_ROPE], F32R, kind="ExternalInput")
    kvbk = nc.dram_tensor("kvbk", [CKV, HG * D_NOPE], F32R, kind="ExternalInput")
    kvbv = nc.dram_tensor("kvbv", [CKV, HG * D_V], F32R, kind="ExternalInput")
    ow = nc.dram_tensor("ow", [HG * D_V, HID], F32R, kind="ExternalInput")
    cos_s = nc.dram_tensor("cos_s", [S_SH, D_ROPE], F32, kind="ExternalInput")
    sin_sg = nc.dram_tensor("sin_sg", [S_SH, D_ROPE], F32, kind="ExternalInput")
    cos2t = nc.dram_tensor("cos2t", [128, S], F32, kind="ExternalInput")
    sin2tg = nc.dram_tensor("sin2tg", [128, S], F32, kind="ExternalInput")
    masks = nc.dram_tensor("masks", [512, 512], F32, kind="ExternalInput")
    ones_col = nc.dram_tensor("ones_col", [128, 1], F32R, kind="ExternalInput")
    ones_row = nc.dram_tensor("ones_row", [1, 128], F32R, kind="ExternalInput")
    out = nc.dram_tensor("out", [S, HID], F32, kind="ExternalOutput")

    with tile.TileContext(nc) as tc, ExitStack() as top:
        const = top.enter_context(tc.tile_pool(name="const", bufs=1))
        dram = top.enter_context(tc.tile_pool(name="dram", bufs=1, space="DRAM"))
        ps_proj = top.enter_context(tc.tile_pool(name="ps_proj", bufs=2, space="PSUM"))
        ps_sc = top.enter_context(tc.tile_pool(name="ps_sc", bufs=2, space="PSUM"))
        ps_ao = top.enter_context(tc.tile_pool(name="ps_ao", bufs=2, space="PSUM"))
        ps_sm = top.enter_context(tc.tile_pool(name="ps_sm", bufs=2, space="PSUM"))

        # ---- constants in SBUF ----
        ident = const.tile([128, 128], F32, tag="ident")
        make_identity(nc, ident[:])
        masks_sb = const.tile([128, 4, 512], F32, tag="masks")
        for m in range(4):
            nc.sync.dma_start(masks_sb[:, m, :], masks.ap()[m * 128:(m + 1) * 128, :])
        cos_s_sb = const.tile([S_SH, D_ROPE], F32, tag="coss")
        sin_sg_sb = const.tile([S_SH, D_ROPE], F32, tag="sinsg")
        nc.sync.dma_start(cos_s_sb[:], cos_s.ap())
        nc.sync.dma_start(sin_sg_sb[:], sin_sg.ap())
        cos2t_sb = const.tile([128, S], F32, tag="cos2t")
        sin2tg_sb = const.tile([128, S], F32, tag="sin2tg")
        nc.sync.dma_start(cos2t_sb[:], cos2t.ap())
        nc.sync.dma_start(sin2tg_sb[:], sin2tg.ap())
        ones_col_sb = const.tile([128, 1], F32R, tag="onesc")
        ones_row_sb = const.tile([1, 128], F32R, tag="onesr")
        nc.sync.dma_start(ones_col_sb[:], ones_col.ap())
        nc.sync.dma_start(ones_row_sb[:], ones_row.ap())

        agi = dram.tile([CA, S_SH], F32R, tag="agi")
        ago = dram.tile([CA * N_CORES, S_SH], F32R, tag="ago")
        outs_d = dram.tile([HG * D_V, S], F32R, tag="outs")

        # ================= Phase A: fused a-proj + rmsnorm + kpe rope ======
        with ExitStack() as pa:
            sba = pa.enter_context(tc.tile_pool(name="sba", bufs=1))
            sbw = pa.enter_context(tc.tile_pool(name="sbw", bufs=4))
            sbt = pa.enter_context(tc.tile_pool(name="sbt", bufs=2))

            xT_sb = sba.tile([128, CC_A, S_SH], F32R, tag="xT")
            for cc in range(CC_A):
                nc.sync.dma_start(xT_sb[:, cc, :], xT.ap()[cc * 128:(cc + 1) * 128, :])
            acts = sba.tile([S_SH, CA], F32, tag="acts")
            for (d0, dn) in NT_A:
                psum = ps_proj.tile([128, 512], F32, tag="proj")
                for cc in range(CC_A):
                    wt = sbw.tile([128, 512], F32R, tag="wa")
                    nc.sync.dma_start(wt[:, :dn], wa.ap()[cc * 128:(cc + 1) * 128, d0:d0 + dn])
                    nc.tensor.matmul(psum[:, :dn], xT_sb[:, cc, :], wt[:, :dn],
                                     start=(cc == 0), stop=(cc == CC_A - 1))
                nc.scalar.copy(acts[:, d0:d0 + dn], psum[:, :dn])

            # rmsnorm factors for qc (cols 0:1536) and ckv (cols 1536:2048)
            sq = sba.tile([S_SH, CQ + CKV], F32, tag="sq")
            nc.vector.tensor_mul(sq[:], acts[:, 0:CQ + CKV], acts[:, 0:CQ + CKV])
            fq = sbt.tile([S_SH, 1], F32, tag="fq")
            fk = sbt.tile([S_SH, 1], F32, tag="fk")
            nc.vector.reduce_sum(fq[:], sq[:, 0:CQ], axis=mybir.AxisListType.X)
            nc.vector.reduce_sum(fk[:], sq[:, CQ:CQ + CKV], axis=mybir.AxisListType.X)
            nc.vector.tensor_scalar(fq[:], fq[:], 1.0 / CQ, EPS, OP.mult, OP.add)
            nc.vector.tensor_scalar(fk[:], fk[:], 1.0 / CKV, EPS, OP.mult, OP.add)
            nc.vector.reciprocal(fq[:], fq[:])
            nc.vector.reciprocal(fk[:], fk[:])
            nc.scalar.activation(fq[:], fq[:], AF.Sqrt)
            nc.scalar.activation(fk[:], fk[:], AF.Sqrt)
            nc.vector.tensor_scalar_mul(acts[:, 0:CQ], acts[:, 0:CQ], fq[:])
            nc.vector.tensor_scalar_mul(acts[:, CQ:CQ + CKV], acts[:, CQ:CQ + CKV], fk[:])

            # k_pe rope (natural [s, 64] layout), cols 2048:2112
            kp0 = CQ + CKV
            kv1 = sbt.tile([S_SH, D_ROPE], F32, tag="kv1")
            kv2 = sbt.tile([S_SH, D_ROPE], F32, tag="kv2")
            nc.vector.tensor_mul(kv1[:], acts[:, kp0:kp0 + 64], cos_s_sb[:])
            nc.vector.tensor_mul(kv2[:, 0:32], acts[:, kp0 + 32:kp0 + 64], sin_sg_sb[:, 0:32])
            nc.vector.tensor_mul(kv2[:, 32:64], acts[:, kp0:kp0 + 32], sin_sg_sb[:, 32:64])
            nc.vector.tensor_add(acts[:, kp0:kp0 + 64], kv1[:], kv2[:])

            # transpose all 17 chunks -> bounce [2112, 128]
            bT = sba.tile([128, 17 * 128], F32R, tag="bT")
            for t in range(17):
                w = 128 if t < 16 else 64
                pt = ps_proj.tile([128, 512], F32, tag="proj")
                nc.tensor.transpose(pt[:w, 0:128], acts[:, t * 128:t * 128 + w], ident[:])
                nc.scalar.copy(bT[:w, t * 128:(t + 1) * 128], pt[:w, 0:128])
                nc.sync.dma_start(agi[t * 128:t * 128 + w, :], bT[:w, t * 128:(t + 1) * 128])

        nc.gpsimd.collective_compute(
            "AllGather", OP.bypass,
            replica_groups=[list(range(N_CORES))],
            ins=[agi.opt()], outs=[ago.opt()],
        )

        # ================= Phase B: per-head-group projections + attention ==
        with ExitStack() as pb:
            sbg = pb.enter_context(tc.tile_pool(name="sbg", bufs=1))
            sbwq = pb.enter_context(tc.tile_pool(name="sbwq", bufs=1))
            sbh = pb.enter_context(tc.tile_pool(name="sbh", bufs=2))
            sbp = pb.enter_context(tc.tile_pool(name="sbp", bufs=1))
            sbv = pb.enter_context(tc.tile_pool(name="sbv", bufs=2))
            sbs = pb.enter_context(tc.tile_pool(name="sbs", bufs=3))

            # gathered activations, stitched per 512-wide s-tile
            qct = []
            ckv = []
            for st in range(2):
                q_t = sbg.tile([128, CQ // 128, 512], F32R, tag=f"qct{st}")
                k_t = sbg.tile([128, CKV // 128, 512], F32R, tag=f"ckv{st}")
                for r in range(4):
                    core = st * 4 + r
                    base = core * CA
                    for c in range(CQ // 128):
                        nc.sync.dma_start(
                            q_t[:, c, r * 128:(r + 1) * 128],
                            ago[base + c * 128:base + (c + 1) * 128, :])
                    for c in range(CKV // 128):
                        nc.sync.dma_start(
                            k_t[:, c, r * 128:(r + 1) * 128],
                            ago[base + CQ + c * 128:base + CQ + (c + 1) * 128, :])
                qct.append(q_t)
                ckv.append(k_t)
            kpe2 = sbg.tile([128, S], F32R, tag="kpe2")
            for core in range(N_CORES):
                base = core * CA + CQ + CKV
                nc.sync.dma_start(kpe2[0:64, core * 128:(core + 1) * 128],
                                  ago[base:base + 64, :])
                nc.sync.dma_start(kpe2[64:128, core * 128:(core + 1) * 128],
                                  ago[base:base + 64, :])

            for g in range(N_GROUPS):
                h0 = g * G_HEADS
                # --- group weight tiles (one 3-D tile per weight) ---
                qbnw = sbwq.tile([128, CQ // 128, G_HEADS * 128], F32R, tag="qbnw")
                qbpw = sbwq.tile([128, CQ // 128, G_HEADS * 64], F32R, tag="qbpw")
                kvbkw = sbwq.tile([128, CKV // 128, G_HEADS * 128], F32R, tag="kvbkw")
                kvbvw = sbwq.tile([128, CKV // 128, G_HEADS * 128], F32R, tag="kvbvw")
                for c in range(CQ // 128):
                    nc.sync.dma_start(qbnw[:, c, :], qbn.ap()[c * 128:(c + 1) * 128,
                                                              h0 * 128:(h0 + G_HEADS) * 128])
                    nc.sync.dma_start(qbpw[:, c, :], qbp.ap()[c * 128:(c + 1) * 128,
                                                              h0 * 64:(h0 + G_HEADS) * 64])
                for c in range(CKV // 128):
                    nc.sync.dma_start(kvbkw[:, c, :], kvbk.ap()[c * 128:(c + 1) * 128,
                                                                h0 * 128:(h0 + G_HEADS) * 128])
                    nc.sync.dma_start(kvbvw[:, c, :], kvbv.ap()[c * 128:(c + 1) * 128,
                                                                h0 * 128:(h0 + G_HEADS) * 128])

                # --- qT_nope / kT_nope per head; qT_pe pair; v ---
                qTn = []
                kTn = []
                for i in range(G_HEADS):
                    qt_t = sbh.tile([128, S], F32R, tag="qTn")
                    for st in range(2):
                        psum = ps_proj.tile([128, 512], F32, tag="proj")
                        for c in range(CQ // 128):
                            nc.tensor.matmul(psum[:], qbnw[:, c, i * 128:(i + 1) * 128],
                                             qct[st][:, c, :],
                                             start=(c == 0), stop=(c == CQ // 128 - 1))
                        nc.vector.tensor_copy(qt_t[:, st * 512:(st + 1) * 512], psum[:])
                    qTn.append(qt_t)
                    kt_t = sbh.tile([128, S], F32R, tag="kTn")
                    for st in range(2):
                        psum = ps_proj.tile([128, 512], F32, tag="proj")
                        for c in range(CKV // 128):
                            nc.tensor.matmul(psum[:], kvbkw[:, c, i * 128:(i + 1) * 128],
                                             ckv[st][:, c, :],
                                             start=(c == 0), stop=(c == CKV // 128 - 1))
                        nc.vector.tensor_copy(kt_t[:, st * 512:(st + 1) * 512], psum[:])
                    kTn.append(kt_t)

                qp_raw = sbp.tile([128, S], F32, tag="qp_raw")
                for st in range(2):
                    psum = ps_proj.tile([128, 512], F32, tag="proj")
                    for c in range(CQ // 128):
                        nc.tensor.matmul(psum[:], qbpw[:, c, :], qct[st][:, c, :],
                                         start=(c == 0), stop=(c == CQ // 128 - 1))
                    nc.vector.tensor_copy(qp_raw[:, st * 512:(st + 1) * 512], psum[:])
                # rope on the head-pair tile: rows [0:64]=head h0, [64:128]=h0+1
                qTp = sbh.tile([128, S], F32R, tag="qTp")
                rm = sbp.tile([128, S], F32, tag="ropem")
                rs = sbp.tile([128, S], F32, tag="ropes")
                nc.vector.tensor_mul(rm[:], qp_raw[:], cos2t_sb[:])
                # rs = swap32(qp_raw), then multiply by the sign-baked sin table
                for b in range(4):
                    r0 = b * 32
                    r1 = r0 + 32 if b % 2 == 0 else r0 - 32
                    nc.vector.tensor_copy(rs[r0:r0 + 32, :], qp_raw[r1:r1 + 32, :])
                nc.vector.tensor_mul(rs[:], rs[:], sin2tg_sb[:])
                nc.vector.tensor_add(qTp[:], rm[:], rs[:])

                v_g = sbv.tile([128, 8, G_HEADS * 128], F32R, tag="v_g")
                for sc in range(8):
                    st = sc // 4
                    psum = ps_proj.tile([128, 512], F32, tag="proj")
                    nn = G_HEADS * 128
                    for c in range(CKV // 128):
                        nc.tensor.matmul(
                            psum[:, :nn],
                            ckv[st][:, c, (sc % 4) * 128:(sc % 4 + 1) * 128],
                            kvbvw[:, c, :],
                            start=(c == 0), stop=(c == CKV // 128 - 1))
                    nc.vector.tensor_copy(v_g[:, sc, :], psum[:, :nn])

                # --- attention for each head in the group ---
                for i in range(G_HEADS):
                    outT = sbh.tile([128, S], F32R, tag="outT")
                    for qt in range(QT):
                        kmax = 4 * (qt + 1)
                        psum_o = ps_ao.tile([128, 512], F32, tag="o")
                        sums = sbs.tile([128, 512], F32R, tag="sums")
                        for kc in range(kmax):
                            ps = ps_sc.tile([128, 512], F32, tag="s")
                            nc.tensor.matmul(ps[:], kTn[i][:, kc * 128:(kc + 1) * 128],
                                             qTn[i][:, qt * 512:(qt + 1) * 512],
                                             start=True, stop=False)
                            b = i * 64
                            nc.tensor.matmul(ps[:], kpe2[b:b + 64, kc * 128:(kc + 1) * 128],
                                             qTp[b:b + 64, qt * 512:(qt + 1) * 512],
                                             start=False, stop=True)
                            pt = sbs.tile([128, 512], F32R, tag="pt")
                            nc.scalar.activation(pt[:], ps[:], AF.Exp, scale=SCALE)
                            if kc >= 4 * qt:
                                nc.vector.tensor_mul(pt[:], pt[:],
                                                     masks_sb[:, kc - 4 * qt, :])
                            if kc == 0:
                                nc.vector.tensor_copy(sums[:], pt[:])
                            else:
                                nc.vector.tensor_add(sums[:], sums[:], pt[:])
                            nc.tensor.matmul(psum_o[:],
                                             v_g[:, kc, i * 128:(i + 1) * 128], pt[:],
                                             start=(kc == 0), stop=(kc == kmax - 1))
                        pss = ps_sm.tile([128, 512], F32, tag="sm")
                        nc.tensor.matmul(pss[0:1, :], ones_col_sb[:], sums[:],
                                         start=True, stop=True)
                        rec = sbs.tile([1, 512], F32R, tag="rec")
                        with nc.allow_low_precision(reason="softmax recip in f32r"):
                            nc.vector.reciprocal(rec[:], pss[0:1, :])
                        psb = ps_sm.tile([128, 512], F32, tag="sm")
                        nc.tensor.matmul(psb[:], ones_row_sb[:], rec[:],
                                         start=True, stop=True)
                        bsb = sbs.tile([128, 512], F32, tag="bsb")
                        nc.vector.tensor_copy(bsb[:], psb[:])
                        nc.vector.tensor_mul(outT[:, qt * 512:(qt + 1) * 512],
                                             psum_o[:], bsb[:])
                    h_glob = h0 + i
                    nc.sync.dma_start(outs_d[h_glob * 128:(h_glob + 1) * 128, :], outT[:])

        # ================= Phase C: partial output projection ===============
        with ExitStack() as pc:
            sbo = pc.enter_context(tc.tile_pool(name="sbo", bufs=1))
            sbow = pc.enter_context(tc.tile_pool(name="sbow", bufs=18))
            sbos = pc.enter_context(tc.tile_pool(name="sbos", bufs=3))
            sbol = pc.enter_context(tc.tile_pool(name="sbol", bufs=20))
            for nt in range(HID // 512):
                owt = []
                for hc in range(HG):
                    t = sbow.tile([128, 512], F32R, tag="ow")
                    nc.sync.dma_start(t[:], ow.ap()[hc * 128:(hc + 1) * 128,
                                                    nt * 512:(nt + 1) * 512])
                    owt.append(t)
                for st in range(8):
                    lhs = []
                    for hc in range(HG):
                        lt = sbol.tile([128, 128], F32R, tag="ol")
                        nc.sync.dma_start(lt[:], outs_d[hc * 128:(hc + 1) * 128,
                                                        st * 128:(st + 1) * 128])
                        lhs.append(lt)
                    psum = ps_proj.tile([128, 512], F32, tag="proj")
                    for hc in range(HG):
                        nc.tensor.matmul(psum[:], lhs[hc][:], owt[hc][:],
                                         start=(hc == 0), stop=(hc == HG - 1))
                    osb = sbos.tile([128, 512], F32, tag="osb")
                    nc.scalar.copy(osb[:], psum[:])
                    nc.sync.dma_start(out.ap()[st * 128:(st + 1) * 128,
                                               nt * 512:(nt + 1) * 512], osb[:])

    nc.compile()
    return nc


def _host_inputs(hidden_states, position_ids, q_a_weight, q_a_layernorm_weight,
                 q_b_weight, kv_a_weight, kv_a_layernorm_weight, kv_b_weight,
                 o_weight):
    x = np.asarray(hidden_states, np.float32).reshape(S, HID)
    pos = np.asarray(position_ids, np.float64).reshape(S)
    q_a_w = np.asarray(q_a_weight, np.float32)
    q_ln = np.asarray(q_a_layernorm_weight, np.float32)
    q_b_w = np.asarray(q_b_weight, np.float32)
    kv_a_w = np.asarray(kv_a_weight, np.float32)
    kv_ln = np.asarray(kv_a_layernorm_weight, np.float32)
    kv_b_w = np.asarray(kv_b_weight, np.float32)
    o_w = np.asarray(o_weight, np.float32)

    wa = np.concatenate([q_a_w, kv_a_w], axis=1)           # [HID, 2112]
    xT = np.ascontiguousarray(x.T)                          # [HID, S]

    # fold the rms-norm weights into the b-projections
    qb = (q_ln[:, None] * q_b_w).reshape(CQ, H, D_Q)
    kvb = (kv_ln[:, None] * kv_b_w).reshape(CKV, H, D_NOPE + D_V)

    # rope tables
    inv_freq = 1.0 / (10000.0 ** (np.arange(0, D_ROPE, 2, dtype=np.float64) / D_ROPE))
    freqs = pos[:, None] * inv_freq[None, :]                # [S, 32]
    emb = np.concatenate([freqs, freqs], axis=-1)           # [S, 64]
    cos = np.cos(emb).astype(np.float32)
    sin = np.sin(emb).astype(np.float32)
    sin_sg = np.concatenate([-sin[:, :32], sin[:, 32:]], axis=1)  # [S, 64]
    cosT = np.ascontiguousarray(cos.T)                      # [64, S]
    sinT_sg = np.ascontiguousarray(sin_sg.T)                # [64, S]
    cos2t = np.concatenate([cosT, cosT], axis=0)            # [128, S]
    sin2tg = np.concatenate([sinT_sg, sinT_sg], axis=0)     # [128, S]

    # causal masks for the 4 diagonal offsets
    masks = np.zeros((4, 128, 512), np.float32)
    i = np.arange(128)[:, None]
    j = np.arange(512)[None, :]
    for m in range(4):
        masks[m] = ((i + m * 128) <= j).astype(np.float32)
    masks = masks.reshape(512, 512)

    ones_col = np.ones((128, 1), np.float32)
    ones_row = np.ones((1, 128), np.float32)

    in_maps = []
    for c in range(N_CORES):
        hs = slice(c * HG, (c + 1) * HG)
        in_maps.append({
            "xT": np.ascontiguousarray(xT[:, c * S_SH:(c + 1) * S_SH]),
            "wa": wa,
            "qbn": np.ascontiguousarray(qb[:, hs, :D_NOPE].reshape(CQ, HG * D_NOPE)),
            "qbp": np.ascontiguousarray(qb[:, hs, D_NOPE:].reshape(CQ, HG * D_ROPE)),
            "kvbk": np.ascontiguousarray(kvb[:, hs, :D_NOPE].reshape(CKV, HG * D_NOPE)),
            "kvbv": np.ascontiguousarray(kvb[:, hs, D_NOPE:].reshape(CKV, HG * D_V)),
            "ow": np.ascontiguousarray(o_w[c * HG * D_V:(c + 1) * HG * D_V, :]),
            "cos_s": np.ascontiguousarray(cos[c * S_SH:(c + 1) * S_SH, :]),
            "sin_sg": np.ascontiguousarray(sin_sg[c * S_SH:(c + 1) * S_SH, :]),
            "cos2t": cos2t,
            "sin2tg": sin2tg,
            "masks": masks,
            "ones_col": ones_col,
            "ones_row": ones_row,
        })
    return in_maps


def kernel(**inputs):
    global LAST_EXEC_NS
    trace = bool(inputs.pop("_trace", False))
    in_maps = _host_inputs(**inputs)
    if "nc" not in _CACHE:
        _CACHE["nc"] = _build_nc()
    nc = _CACHE["nc"]
    res = bass_utils.run_bass_kernel_spmd(
        nc, in_maps, core_ids=list(range(N_CORES)), trace=trace)
    LAST_EXEC_NS = res.exec_time_ns
    total = np.zeros((S, HID), np.float64)
    for c in range(N_CORES):
        total += res.results[c]["out"].astype(np.float64)
    return total.astype(np.float32).reshape(1, 1, S, HID)



# revision 7
# speedup vs baseline: 2.1910x; 2.1910x over previous
"""DeepseekV3 MLA attention prefill (S=1024, H=128 heads, HID=7168) on 8 TRN2
NeuronCores.

Sharding: tensor-parallel over heads (16 heads/core) for q_b/kv_b/attention/
o_proj; the low-rank input projections (q_a / kv_a) are sequence-sharded
(128 rows/core) and exchanged with one small AllGather of the transposed,
rms-normed activations. Each core emits a partial output projection
(contraction over its own 16 heads); the host sums the 8 partials.

All matmuls run in bf16 (fp32r draws enough PE power to trip EDPP duty
throttling to 50%; bf16 runs at the same 1 cycle/row without it and halves
weight DMA + LDWEIGHTS traffic). PSUM accumulation and softmax statistics
stay float32.
"""
import math
import numpy as np
import ml_dtypes

import concourse.bass as bass
import concourse.mybir as mybir
import concourse.bacc as bacc
import concourse.tile as tile
import concourse.bass_utils as bass_utils
from concourse.masks import make_identity
from contextlib import ExitStack

F32 = mybir.dt.float32
F32R = mybir.dt.float32r
BF16 = mybir.dt.bfloat16
AF = mybir.ActivationFunctionType
OP = mybir.AluOpType

N_CORES = 8
S = 1024
HID = 7168
H = 128
HG = H // N_CORES          # 16 heads per core
D_NOPE = 128
D_ROPE = 64
D_Q = D_NOPE + D_ROPE      # 192
D_V = 128
CQ = 1536                  # q lora rank
CKV = 512                  # kv lora rank
CA = CQ + CKV + D_ROPE     # 2112 fused a-proj cols
S_SH = S // N_CORES        # 128 sequence rows per core
CC_A = HID // 128          # 56 contraction chunks for a-proj
NT_A = [(0, 512), (512, 512), (1024, 512), (1536, 512), (2048, 64)]
SCALE = 1.0 / math.sqrt(D_Q)
EPS = 1e-6
G_HEADS = 2                # heads per group
N_GROUPS = HG // G_HEADS   # 8 groups
QT = 2                     # q-tiles of 512 per head
LAST_EXEC_NS = None

_CACHE = {}


def _build_nc():
    nc = bacc.Bacc("TRN2", target_bir_lowering=False, debug=False,
                   num_devices=N_CORES)

    xT = nc.dram_tensor("xT", [HID, S_SH], BF16, kind="ExternalInput")
    wa = nc.dram_tensor("wa", [HID, CA], BF16, kind="ExternalInput")
    qbn = nc.dram_tensor("qbn", [CQ, HG * D_NOPE], BF16, kind="ExternalInput")
    qbp = nc.dram_tensor("qbp", [CQ, HG * D_ROPE], BF16, kind="ExternalInput")
    kvbk = nc.dram_tensor("kvbk", [CKV, HG * D_NOPE], BF16, kind="ExternalInput")
    kvbv = nc.dram_tensor("kvbv", [CKV, HG * D_V], BF16, kind="ExternalInput")
    ow = nc.dram_tensor("ow", [HG * D_V, HID], BF16, kind="ExternalInput")
    cos_s = nc.dram_tensor("cos_s", [S_SH, D_ROPE], F32, kind="ExternalInput")
    sin_sg = nc.dram_tensor("sin_sg", [S_SH, D_ROPE], F32, kind="ExternalInput")
    cos2t = nc.dram_tensor("cos2t", [128, S], BF16, kind="ExternalInput")
    sin2tg = nc.dram_tensor("sin2tg", [128, S], BF16, kind="ExternalInput")
    masks = nc.dram_tensor("masks", [512, 512], BF16, kind="ExternalInput")
    ones_col = nc.dram_tensor("ones_col", [128, 1], F32R, kind="ExternalInput")
    ones_row = nc.dram_tensor("ones_row", [1, 128], F32R, kind="ExternalInput")
    out = nc.dram_tensor("out", [S, HID], F32, kind="ExternalOutput")

    with tile.TileContext(nc) as tc, ExitStack() as top:
        const = top.enter_context(tc.tile_pool(name="const", bufs=1))
        outsp = top.enter_context(tc.tile_pool(name="outsp", bufs=1))
        dram = top.enter_context(tc.tile_pool(name="dram", bufs=1, space="DRAM"))
        # ---- constants in SBUF ----
        ident = const.tile([128, 128], BF16, tag="ident")
        make_identity(nc, ident[:])
        masks_sb = const.tile([128, 4, 512], BF16, tag="masks")
        for m in range(4):
            nc.sync.dma_start(masks_sb[:, m, :], masks.ap()[m * 128:(m + 1) * 128, :])
        cos_s_sb = const.tile([S_SH, D_ROPE], F32, tag="coss")
        sin_sg_sb = const.tile([S_SH, D_ROPE], F32, tag="sinsg")
        nc.sync.dma_start(cos_s_sb[:], cos_s.ap())
        nc.sync.dma_start(sin_sg_sb[:], sin_sg.ap())
        cos2t_sb = const.tile([128, S], BF16, tag="cos2t")
        sin2tg_sb = const.tile([128, S], BF16, tag="sin2tg")
        nc.sync.dma_start(cos2t_sb[:], cos2t.ap())
        nc.sync.dma_start(sin2tg_sb[:], sin2tg.ap())
        ones_col_sb = const.tile([128, 1], F32R, tag="onesc")
        ones_row_sb = const.tile([1, 128], F32R, tag="onesr")
        nc.sync.dma_start(ones_col_sb[:], ones_col.ap())
        nc.sync.dma_start(ones_row_sb[:], ones_row.ap())

        # attention outputs for all 16 local heads, [dv, head, s], bf16
        outs_all = outsp.tile([128, HG, S], BF16, tag="outs_all")

        agi = dram.tile([CA, S_SH], BF16, tag="agi")
        ago = dram.tile([CA * N_CORES, S_SH], BF16, tag="ago")

        # ================= Phase A: fused a-proj + rmsnorm + kpe rope ======
        with ExitStack() as pa:
            sba = pa.enter_context(tc.tile_pool(name="sba", bufs=1))
            sbw = pa.enter_context(tc.tile_pool(name="sbw", bufs=3))
            sbt = pa.enter_context(tc.tile_pool(name="sbt", bufs=2))
            ps_a = pa.enter_context(tc.tile_pool(name="ps_a", bufs=1, space="PSUM"))
            ps_tp = pa.enter_context(tc.tile_pool(name="ps_tp", bufs=2, space="PSUM"))

            xT_sb = sba.tile([128, CC_A, S_SH], BF16, tag="xT")
            for cc in range(CC_A):
                nc.sync.dma_start(xT_sb[:, cc, :], xT.ap()[cc * 128:(cc + 1) * 128, :])
            # 5 live psum banks accumulate the full [128, 2112] activation row
            pa_t = [ps_a.tile([128, 512], F32, name=f"pa{nt}", tag=f"pa{nt}")
                    for nt in range(5)]
            for cc in range(CC_A):
                wt = sbw.tile([128, CA], BF16, tag="wa")
                nc.sync.dma_start(wt[:], wa.ap()[cc * 128:(cc + 1) * 128, :])
                for nt, (d0, dn) in enumerate(NT_A):
                    nc.tensor.matmul(pa_t[nt][:, :dn], xT_sb[:, cc, :],
                                     wt[:, d0:d0 + dn],
                                     start=(cc == 0), stop=(cc == CC_A - 1))
            acts = sba.tile([S_SH, CA], F32, tag="acts")
            for nt, (d0, dn) in enumerate(NT_A):
                nc.scalar.copy(acts[:, d0:d0 + dn], pa_t[nt][:, :dn])

            # rmsnorm factors for qc (cols 0:1536) and ckv (cols 1536:2048)
            sq = sba.tile([S_SH, CQ + CKV], F32, tag="sq")
            nc.vector.tensor_mul(sq[:], acts[:, 0:CQ + CKV], acts[:, 0:CQ + CKV])
            fq = sbt.tile([S_SH, 1], F32, tag="fq")
            fk = sbt.tile([S_SH, 1], F32, tag="fk")
            nc.vector.reduce_sum(fq[:], sq[:, 0:CQ], axis=mybir.AxisListType.X)
            nc.vector.reduce_sum(fk[:], sq[:, CQ:CQ + CKV], axis=mybir.AxisListType.X)
            nc.vector.tensor_scalar(fq[:], fq[:], 1.0 / CQ, EPS, OP.mult, OP.add)
            nc.vector.tensor_scalar(fk[:], fk[:], 1.0 / CKV, EPS, OP.mult, OP.add)
            nc.vector.reciprocal(fq[:], fq[:])
            nc.vector.reciprocal(fk[:], fk[:])
            nc.scalar.activation(fq[:], fq[:], AF.Sqrt)
            nc.scalar.activation(fk[:], fk[:], AF.Sqrt)
            acts_bf = sba.tile([S_SH, CA], BF16, tag="acts_bf")
            nc.vector.tensor_scalar_mul(acts_bf[:, 0:CQ], acts[:, 0:CQ], fq[:])
            nc.vector.tensor_scalar_mul(acts_bf[:, CQ:CQ + CKV],
                                        acts[:, CQ:CQ + CKV], fk[:])

            # k_pe rope (natural [s, 64] layout), cols 2048:2112
            kp0 = CQ + CKV
            kv1 = sbt.tile([S_SH, D_ROPE], F32, tag="kv1")
            kv2 = sbt.tile([S_SH, D_ROPE], F32, tag="kv2")
            nc.vector.tensor_mul(kv1[:], acts[:, kp0:kp0 + 64], cos_s_sb[:])
            nc.vector.tensor_mul(kv2[:, 0:32], acts[:, kp0 + 32:kp0 + 64], sin_sg_sb[:, 0:32])
            nc.vector.tensor_mul(kv2[:, 32:64], acts[:, kp0:kp0 + 32], sin_sg_sb[:, 32:64])
            nc.vector.tensor_add(acts_bf[:, kp0:kp0 + 64], kv1[:], kv2[:])

            # transpose all 17 chunks -> bounce [2112, 128] bf16
            bT = sba.tile([128, 17 * 128], BF16, tag="bT")
            for t in range(17):
                w = 128 if t < 16 else 64
                pt = ps_tp.tile([128, 512], BF16, tag="projbf")
                nc.tensor.transpose(pt[:w, 0:128], acts_bf[:, t * 128:t * 128 + w],
                                    ident[:])
                nc.scalar.copy(bT[:w, t * 128:(t + 1) * 128], pt[:w, 0:128])
                nc.sync.dma_start(agi[t * 128:t * 128 + w, :], bT[:w, t * 128:(t + 1) * 128])

        ps_proj = top.enter_context(tc.tile_pool(name="ps_proj", bufs=2, space="PSUM"))

        nc.gpsimd.collective_compute(
            "AllGather", OP.bypass,
            replica_groups=[list(range(N_CORES))],
            ins=[agi.opt()], outs=[ago.opt()],
        )

        # ================= Phase B: per-head-group projections + attention ==
        with ExitStack() as pb:
            ps_sc = pb.enter_context(tc.tile_pool(name="ps_sc", bufs=2, space="PSUM"))
            ps_ao = pb.enter_context(tc.tile_pool(name="ps_ao", bufs=2, space="PSUM"))
            ps_sm = pb.enter_context(tc.tile_pool(name="ps_sm", bufs=2, space="PSUM"))
            sbg = pb.enter_context(tc.tile_pool(name="sbg", bufs=1))
            sbwq = pb.enter_context(tc.tile_pool(name="sbwq", bufs=2))
            sbh = pb.enter_context(tc.tile_pool(name="sbh", bufs=2))
            sbp = pb.enter_context(tc.tile_pool(name="sbp", bufs=2))
            sbv = pb.enter_context(tc.tile_pool(name="sbv", bufs=2))
            sbs = pb.enter_context(tc.tile_pool(name="sbs", bufs=3))

            # gathered activations, stitched per 512-wide s-tile
            qct = []
            ckv = []
            for st in range(2):
                q_t = sbg.tile([128, CQ // 128, 512], BF16, tag=f"qct{st}")
                k_t = sbg.tile([128, CKV // 128, 512], BF16, tag=f"ckv{st}")
                for r in range(4):
                    core = st * 4 + r
                    base = core * CA
                    for c in range(CQ // 128):
                        nc.sync.dma_start(
                            q_t[:, c, r * 128:(r + 1) * 128],
                            ago[base + c * 128:base + (c + 1) * 128, :])
                    for c in range(CKV // 128):
                        nc.sync.dma_start(
                            k_t[:, c, r * 128:(r + 1) * 128],
                            ago[base + CQ + c * 128:base + CQ + (c + 1) * 128, :])
                qct.append(q_t)
                ckv.append(k_t)
            kpe2 = sbg.tile([128, S], BF16, tag="kpe2")
            for core in range(N_CORES):
                base = core * CA + CQ + CKV
                nc.sync.dma_start(kpe2[0:64, core * 128:(core + 1) * 128],
                                  ago[base:base + 64, :])
                nc.sync.dma_start(kpe2[64:128, core * 128:(core + 1) * 128],
                                  ago[base:base + 64, :])

            for g in range(N_GROUPS):
                h0 = g * G_HEADS
                # --- group weight tiles (one 3-D tile per weight) ---
                qbnw = sbwq.tile([128, CQ // 128, G_HEADS * 128], BF16, tag="qbnw")
                qbpw = sbwq.tile([128, CQ // 128, G_HEADS * 64], BF16, tag="qbpw")
                kvbkw = sbwq.tile([128, CKV // 128, G_HEADS * 128], BF16, tag="kvbkw")
                kvbvw = sbwq.tile([128, CKV // 128, G_HEADS * 128], BF16, tag="kvbvw")
                for c in range(CQ // 128):
                    nc.sync.dma_start(qbnw[:, c, :], qbn.ap()[c * 128:(c + 1) * 128,
                                                              h0 * 128:(h0 + G_HEADS) * 128])
                    nc.sync.dma_start(qbpw[:, c, :], qbp.ap()[c * 128:(c + 1) * 128,
                                                              h0 * 64:(h0 + G_HEADS) * 64])
                for c in range(CKV // 128):
                    nc.sync.dma_start(kvbkw[:, c, :], kvbk.ap()[c * 128:(c + 1) * 128,
                                                                h0 * 128:(h0 + G_HEADS) * 128])
                    nc.sync.dma_start(kvbvw[:, c, :], kvbv.ap()[c * 128:(c + 1) * 128,
                                                                h0 * 128:(h0 + G_HEADS) * 128])

                # --- qT_nope / kT_nope per head; qT_pe pair; v ---
                qTn = []
                kTn = []
                for i in range(G_HEADS):
                    qt_t = sbh.tile([128, S], BF16, tag="qTn")
                    ps2 = [ps_proj.tile([128, 512], F32, name=f"ps2_{st}", tag="proj")
                           for st in range(2)]
                    for c in range(CQ // 128):
                        for st in range(2):
                            nc.tensor.matmul(ps2[st][:],
                                             qbnw[:, c, i * 128:(i + 1) * 128],
                                             qct[st][:, c, :],
                                             start=(c == 0), stop=(c == CQ // 128 - 1))
                    for st in range(2):
                        nc.vector.tensor_copy(qt_t[:, st * 512:(st + 1) * 512], ps2[st][:])
                    qTn.append(qt_t)
                    kt_t = sbh.tile([128, S], BF16, tag="kTn")
                    ps2 = [ps_proj.tile([128, 512], F32, name=f"ps2_{st}", tag="proj")
                           for st in range(2)]
                    for c in range(CKV // 128):
                        for st in range(2):
                            nc.tensor.matmul(ps2[st][:],
                                             kvbkw[:, c, i * 128:(i + 1) * 128],
                                             ckv[st][:, c, :],
                                             start=(c == 0), stop=(c == CKV // 128 - 1))
                    for st in range(2):
                        nc.vector.tensor_copy(kt_t[:, st * 512:(st + 1) * 512], ps2[st][:])
                    kTn.append(kt_t)

                qp_raw = sbp.tile([128, S], BF16, tag="qp_raw")
                ps2 = [ps_proj.tile([128, 512], F32, name=f"ps2_{st}", tag="proj")
                       for st in range(2)]
                for c in range(CQ // 128):
                    for st in range(2):
                        nc.tensor.matmul(ps2[st][:], qbpw[:, c, :], qct[st][:, c, :],
                                         start=(c == 0), stop=(c == CQ // 128 - 1))
                for st in range(2):
                    nc.scalar.copy(qp_raw[:, st * 512:(st + 1) * 512], ps2[st][:])
                # rope on the head-pair tile: rows [0:64]=head h0, [64:128]=h0+1
                qTp = sbh.tile([128, S], BF16, tag="qTp")
                rm = sbp.tile([128, S], BF16, tag="ropem")
                rs = sbp.tile([128, S], BF16, tag="ropes")
                nc.vector.tensor_mul(rm[:], qp_raw[:], cos2t_sb[:])
                # rs = swap32(qp_raw), then multiply by the sign-baked sin table
                for b in range(4):
                    r0 = b * 32
                    r1 = r0 + 32 if b % 2 == 0 else r0 - 32
                    nc.vector.tensor_copy(rs[r0:r0 + 32, :], qp_raw[r1:r1 + 32, :])
                nc.vector.tensor_mul(rs[:], rs[:], sin2tg_sb[:])
                nc.vector.tensor_add(qTp[:], rm[:], rs[:])

                v_g = sbv.tile([128, 8, G_HEADS * 128], BF16, tag="v_g")
                for sc in range(8):
                    st = sc // 4
                    psum = ps_proj.tile([128, 512], F32, tag="proj")
                    nn = G_HEADS * 128
                    for c in range(CKV // 128):
                        nc.tensor.matmul(
                            psum[:, :nn],
                            ckv[st][:, c, (sc % 4) * 128:(sc % 4 + 1) * 128],
                            kvbvw[:, c, :],
                            start=(c == 0), stop=(c == CKV // 128 - 1))
                    nc.vector.tensor_copy(v_g[:, sc, :], psum[:, :nn])

                # --- attention for each head in the group ---
                for i in range(G_HEADS):
                    h_loc = h0 + i
                    for qt in range(QT):
                        kmax = 4 * (qt + 1)
                        psum_o = ps_ao.tile([128, 512], F32, tag="o")
                        sums = sbs.tile([128, 512], F32R, tag="sums")
                        for kc in range(kmax):
                            ps = ps_sc.tile([128, 512], F32, tag="s")
                            nc.tensor.matmul(ps[:], kTn[i][:, kc * 128:(kc + 1) * 128],
                                             qTn[i][:, qt * 512:(qt + 1) * 512],
                                             start=True, stop=False)
                            b = i * 64
                            nc.tensor.matmul(ps[:], kpe2[b:b + 64, kc * 128:(kc + 1) * 128],
                                             qTp[b:b + 64, qt * 512:(qt + 1) * 512],
                                             start=False, stop=True)
                            pt = sbs.tile([128, 512], BF16, tag="pt")
                            nc.scalar.activation(pt[:], ps[:], AF.Exp, scale=SCALE)
                            if kc >= 4 * qt:
                                nc.vector.tensor_mul(pt[:], pt[:],
                                                     masks_sb[:, kc - 4 * qt, :])
                            if kc == 0:
                                nc.vector.tensor_copy(sums[:], pt[:])
                            else:
                                nc.vector.tensor_add(sums[:], sums[:], pt[:])
                            nc.tensor.matmul(psum_o[:],
                                             v_g[:, kc, i * 128:(i + 1) * 128], pt[:],
                                             start=(kc == 0), stop=(kc == kmax - 1))
                        pss = ps_sm.tile([128, 512], F32, tag="sm")
                        nc.tensor.matmul(pss[0:1, :], ones_col_sb[:], sums[:],
                                         start=True, stop=True)
                        rec = sbs.tile([1, 512], F32R, tag="rec")
                        with nc.allow_low_precision(reason="softmax recip in f32r"):
                            nc.vector.reciprocal(rec[:], pss[0:1, :])
                        psb = ps_sm.tile([128, 512], F32, tag="sm")
                        nc.tensor.matmul(psb[:], ones_row_sb[:], rec[:],
                                         start=True, stop=True)
                        bsb = sbs.tile([128, 512], F32, tag="bsb")
                        nc.scalar.copy(bsb[:], psb[:])
                        nc.vector.tensor_mul(outs_all[:, h_loc, qt * 512:(qt + 1) * 512],
                                             psum_o[:], bsb[:])

        # ================= Phase C: partial output projection ===============
        with ExitStack() as pc:
            sbow = pc.enter_context(tc.tile_pool(name="sbow", bufs=2))
            sbos = pc.enter_context(tc.tile_pool(name="sbos", bufs=3))
            for nt in range(HID // 512):
                owt = sbow.tile([128, HG, 512], BF16, tag="ow")
                for hc in range(HG):
                    nc.sync.dma_start(owt[:, hc, :],
                                      ow.ap()[hc * 128:(hc + 1) * 128,
                                              nt * 512:(nt + 1) * 512])
                for st in range(8):
                    psum = ps_proj.tile([128, 512], F32, tag="proj")
                    for hc in range(HG):
                        nc.tensor.matmul(psum[:],
                                         outs_all[:, hc, st * 128:(st + 1) * 128],
                                         owt[:, hc, :],
                                         start=(hc == 0), stop=(hc == HG - 1))
                    osb = sbos.tile([128, 512], F32, tag="osb")
                    nc.scalar.copy(osb[:], psum[:])
                    nc.sync.dma_start(out.ap()[st * 128:(st + 1) * 128,
                                               nt * 512:(nt + 1) * 512], osb[:])

    nc.compile()
    return nc


def _host_inputs(hidden_states, position_ids, q_a_weight, q_a_layernorm_weight,
                 q_b_weight, kv_a_weight, kv_a_layernorm_weight, kv_b_weight,
                 o_weight):
    bf = ml_dtypes.bfloat16
    x = np.asarray(hidden_states, np.float32).reshape(S, HID)
    pos = np.asarray(position_ids, np.float64).reshape(S)
    q_a_w = np.asarray(q_a_weight, np.float32)
    q_ln = np.asarray(q_a_layernorm_weight, np.float32)
    q_b_w = np.asarray(q_b_weight, np.float32)
    kv_a_w = np.asarray(kv_a_weight, np.float32)
    kv_ln = np.asarray(kv_a_layernorm_weight, np.float32)
    kv_b_w = np.asarray(kv_b_weight, np.float32)
    o_w = np.asarray(o_weight, np.float32)

    wa = np.concatenate([q_a_w, kv_a_w], axis=1).astype(bf)    # [HID, 2112]
    xT = np.ascontiguousarray(x.T).astype(bf)                   # [HID, S]

    # fold the rms-norm weights into the b-projections
    qb = (q_ln[:, None] * q_b_w).reshape(CQ, H, D_Q)
    kvb = (kv_ln[:, None] * kv_b_w).reshape(CKV, H, D_NOPE + D_V)

    # rope tables
    inv_freq = 1.0 / (10000.0 ** (np.arange(0, D_ROPE, 2, dtype=np.float64) / D_ROPE))
    freqs = pos[:, None] * inv_freq[None, :]                # [S, 32]
    emb = np.concatenate([freqs, freqs], axis=-1)           # [S, 64]
    cos = np.cos(emb).astype(np.float32)
    sin = np.sin(emb).astype(np.float32)
    sin_sg = np.concatenate([-sin[:, :32], sin[:, 32:]], axis=1)  # [S, 64]
    cosT = np.ascontiguousarray(cos.T)                      # [64, S]
    sinT_sg = np.ascontiguousarray(sin_sg.T)                # [64, S]
    cos2t = np.concatenate([cosT, cosT], axis=0).astype(bf)     # [128, S]
    sin2tg = np.concatenate([sinT_sg, sinT_sg], axis=0).astype(bf)  # [128, S]

    # causal masks for the 4 diagonal offsets
    masks = np.zeros((4, 128, 512), np.float32)
    i = np.arange(128)[:, None]
    j = np.arange(512)[None, :]
    for m in range(4):
        masks[m] = ((i + m * 128) <= j).astype(np.float32)
    masks = masks.reshape(512, 512).astype(bf)

    ones_col = np.ones((128, 1), np.float32)
    ones_row = np.ones((1, 128), np.float32)

    in_maps = []
    for c in range(N_CORES):
        hs = slice(c * HG, (c + 1) * HG)
        in_maps.append({
            "xT": np.ascontiguousarray(xT[:, c * S_SH:(c + 1) * S_SH]),
            "wa": wa,
            "qbn": np.ascontiguousarray(
                qb[:, hs, :D_NOPE].reshape(CQ, HG * D_NOPE)).astype(bf),
            "qbp": np.ascontiguousarray(
                qb[:, hs, D_NOPE:].reshape(CQ, HG * D_ROPE)).astype(bf),
            "kvbk": np.ascontiguousarray(
                kvb[:, hs, :D_NOPE].reshape(CKV, HG * D_NOPE)).astype(bf),
            "kvbv": np.ascontiguousarray(
                kvb[:, hs, D_NOPE:].reshape(CKV, HG * D_V)).astype(bf),
            "ow": np.ascontiguousarray(
                o_w[c * HG * D_V:(c + 1) * HG * D_V, :]).astype(bf),
            "cos_s": np.ascontiguousarray(cos[c * S_SH:(c + 1) * S_SH, :]),
            "sin_sg": np.ascontiguousarray(sin_sg[c * S_SH:(c + 1) * S_SH, :]),
            "cos2t": cos2t,
            "sin2tg": sin2tg,
            "masks": masks,
            "ones_col": ones_col,
            "ones_row": ones_row,
        })
    return in_maps


def kernel(**inputs):
    global LAST_EXEC_NS
    trace = bool(inputs.pop("_trace", False))
    in_maps = _host_inputs(**inputs)
    if "nc" not in _CACHE:
        _CACHE["nc"] = _build_nc()
    nc = _CACHE["nc"]
    res = bass_utils.run_bass_kernel_spmd(
        nc, in_maps, core_ids=list(range(N_CORES)), trace=trace)
    LAST_EXEC_NS = res.exec_time_ns
    total = np.zeros((S, HID), np.float64)
    for c in range(N_CORES):
        total += res.results[c]["out"].astype(np.float64)
    return total.astype(np.float32).reshape(1, 1, S, HID)


# revision 9
# speedup vs baseline: 2.2474x; 1.0257x over previous
"""DeepseekV3 MLA attention prefill (S=1024, H=128 heads, HID=7168) on 8 TRN2
NeuronCores.

Sharding: tensor-parallel over heads (16 heads/core) for q_b/kv_b/attention/
o_proj; the low-rank input projections (q_a / kv_a) are sequence-sharded
(128 rows/core) and exchanged with one small AllGather of the transposed,
rms-normed activations. Each core emits a partial output projection
(contraction over its own 16 heads); the host sums the 8 partials.

All matmuls run in bf16 (fp32r draws enough PE power to trip EDPP duty
throttling to 50%; bf16 runs at the same 1 cycle/row without it and halves
weight DMA + LDWEIGHTS traffic). PSUM accumulation and softmax statistics
stay float32.
"""
import math
import numpy as np
import ml_dtypes

import concourse.bass as bass
import concourse.mybir as mybir
import concourse.bacc as bacc
import concourse.tile as tile
import concourse.bass_utils as bass_utils
from concourse.masks import make_identity
from contextlib import ExitStack

F32 = mybir.dt.float32
F32R = mybir.dt.float32r
BF16 = mybir.dt.bfloat16
AF = mybir.ActivationFunctionType
OP = mybir.AluOpType

N_CORES = 8
S = 1024
HID = 7168
H = 128
HG = H // N_CORES          # 16 heads per core
D_NOPE = 128
D_ROPE = 64
D_Q = D_NOPE + D_ROPE      # 192
D_V = 128
CQ = 1536                  # q lora rank
CKV = 512                  # kv lora rank
CA = CQ + CKV + D_ROPE     # 2112 fused a-proj cols
S_SH = S // N_CORES        # 128 sequence rows per core
CC_A = HID // 128          # 56 contraction chunks for a-proj
NT_A = [(0, 512), (512, 512), (1024, 512), (1536, 512), (2048, 64)]
SCALE = 1.0 / math.sqrt(D_Q)
EPS = 1e-6
G_HEADS = 2                # heads per group
N_GROUPS = HG // G_HEADS   # 8 groups
QT = 2                     # q-tiles of 512 per head
LAST_EXEC_NS = None

_CACHE = {}


def _scalar_reciprocal(nc, out, in_):
    eng = nc.scalar
    inputs = [eng.lower_ap(in_)]
    for v in (0.0, 1.0, 0.0):  # bias, scale, alpha
        inputs.append(mybir.ImmediateValue(dtype=mybir.dt.float32, value=v))
    return eng.add_instruction(mybir.InstActivation(
        name=eng.bass.get_next_instruction_name(),
        func=AF.Reciprocal, ins=inputs, outs=[eng.lower_ap(out)]))


def _build_nc():
    nc = bacc.Bacc("TRN2", target_bir_lowering=False, debug=False,
                   num_devices=N_CORES)

    xT = nc.dram_tensor("xT", [HID, S_SH], BF16, kind="ExternalInput")
    wa = nc.dram_tensor("wa", [HID, CA], BF16, kind="ExternalInput")
    qbn = nc.dram_tensor("qbn", [CQ, HG * D_NOPE], BF16, kind="ExternalInput")
    qbp = nc.dram_tensor("qbp", [CQ, HG * D_ROPE], BF16, kind="ExternalInput")
    kvbk = nc.dram_tensor("kvbk", [CKV, HG * D_NOPE], BF16, kind="ExternalInput")
    kvbv = nc.dram_tensor("kvbv", [CKV, HG * D_V], BF16, kind="ExternalInput")
    ow = nc.dram_tensor("ow", [HG * D_V, HID], BF16, kind="ExternalInput")
    cos_s = nc.dram_tensor("cos_s", [S_SH, D_ROPE], F32, kind="ExternalInput")
    sin_sg = nc.dram_tensor("sin_sg", [S_SH, D_ROPE], F32, kind="ExternalInput")
    cos2t = nc.dram_tensor("cos2t", [128, S], BF16, kind="ExternalInput")
    sin2tg = nc.dram_tensor("sin2tg", [128, S], BF16, kind="ExternalInput")
    masks = nc.dram_tensor("masks", [512, 512], BF16, kind="ExternalInput")
    ones_col = nc.dram_tensor("ones_col", [128, 1], F32R, kind="ExternalInput")
    ones_row = nc.dram_tensor("ones_row", [1, 128], F32R, kind="ExternalInput")
    out = nc.dram_tensor("out", [S, HID], F32, kind="ExternalOutput")

    with tile.TileContext(nc) as tc, ExitStack() as top:
        const = top.enter_context(tc.tile_pool(name="const", bufs=1))
        outsp = top.enter_context(tc.tile_pool(name="outsp", bufs=1))
        dram = top.enter_context(tc.tile_pool(name="dram", bufs=1, space="DRAM"))
        # ---- constants in SBUF ----
        ident = const.tile([128, 128], BF16, tag="ident")
        make_identity(nc, ident[:])
        masks_sb = const.tile([128, 4, 512], BF16, tag="masks")
        for m in range(4):
            nc.sync.dma_start(masks_sb[:, m, :], masks.ap()[m * 128:(m + 1) * 128, :])
        cos_s_sb = const.tile([S_SH, D_ROPE], F32, tag="coss")
        sin_sg_sb = const.tile([S_SH, D_ROPE], F32, tag="sinsg")
        nc.sync.dma_start(cos_s_sb[:], cos_s.ap())
        nc.sync.dma_start(sin_sg_sb[:], sin_sg.ap())
        cos2t_sb = const.tile([128, S], BF16, tag="cos2t")
        sin2tg_sb = const.tile([128, S], BF16, tag="sin2tg")
        nc.sync.dma_start(cos2t_sb[:], cos2t.ap())
        nc.sync.dma_start(sin2tg_sb[:], sin2tg.ap())
        ones_col_sb = const.tile([128, 1], F32R, tag="onesc")
        ones_row_sb = const.tile([1, 128], F32R, tag="onesr")
        nc.sync.dma_start(ones_col_sb[:], ones_col.ap())
        nc.sync.dma_start(ones_row_sb[:], ones_row.ap())

        # attention outputs for all 16 local heads, [dv, head, s], bf16
        outs_all = outsp.tile([128, HG, S], BF16, tag="outs_all")

        agi = dram.tile([CA, S_SH], BF16, tag="agi")
        ago = dram.tile([CA * N_CORES, S_SH], BF16, tag="ago")

        # ================= Phase A: fused a-proj + rmsnorm + kpe rope ======
        with ExitStack() as pa:
            sba = pa.enter_context(tc.tile_pool(name="sba", bufs=1))
            sbw = pa.enter_context(tc.tile_pool(name="sbw", bufs=3))
            sbt = pa.enter_context(tc.tile_pool(name="sbt", bufs=2))
            ps_a = pa.enter_context(tc.tile_pool(name="ps_a", bufs=1, space="PSUM"))
            ps_tp = pa.enter_context(tc.tile_pool(name="ps_tp", bufs=2, space="PSUM"))

            xT_sb = sba.tile([128, CC_A, S_SH], BF16, tag="xT")
            for cc in range(CC_A):
                nc.sync.dma_start(xT_sb[:, cc, :], xT.ap()[cc * 128:(cc + 1) * 128, :])
            # 5 live psum banks accumulate the full [128, 2112] activation row
            pa_t = [ps_a.tile([128, 512], F32, name=f"pa{nt}", tag=f"pa{nt}")
                    for nt in range(5)]
            for cc in range(CC_A):
                wt = sbw.tile([128, CA], BF16, tag="wa")
                nc.sync.dma_start(wt[:], wa.ap()[cc * 128:(cc + 1) * 128, :])
                for nt, (d0, dn) in enumerate(NT_A):
                    nc.tensor.matmul(pa_t[nt][:, :dn], xT_sb[:, cc, :],
                                     wt[:, d0:d0 + dn],
                                     start=(cc == 0), stop=(cc == CC_A - 1))
            acts = sba.tile([S_SH, CA], F32, tag="acts")
            for nt, (d0, dn) in enumerate(NT_A):
                nc.scalar.copy(acts[:, d0:d0 + dn], pa_t[nt][:, :dn])

            # rmsnorm factors for qc (cols 0:1536) and ckv (cols 1536:2048)
            sq = sba.tile([S_SH, CQ + CKV], F32, tag="sq")
            nc.vector.tensor_mul(sq[:], acts[:, 0:CQ + CKV], acts[:, 0:CQ + CKV])
            fq = sbt.tile([S_SH, 1], F32, tag="fq")
            fk = sbt.tile([S_SH, 1], F32, tag="fk")
            nc.vector.reduce_sum(fq[:], sq[:, 0:CQ], axis=mybir.AxisListType.X)
            nc.vector.reduce_sum(fk[:], sq[:, CQ:CQ + CKV], axis=mybir.AxisListType.X)
            nc.vector.tensor_scalar(fq[:], fq[:], 1.0 / CQ, EPS, OP.mult, OP.add)
            nc.vector.tensor_scalar(fk[:], fk[:], 1.0 / CKV, EPS, OP.mult, OP.add)
            nc.vector.reciprocal(fq[:], fq[:])
            nc.vector.reciprocal(fk[:], fk[:])
            nc.scalar.activation(fq[:], fq[:], AF.Sqrt)
            nc.scalar.activation(fk[:], fk[:], AF.Sqrt)
            acts_bf = sba.tile([S_SH, CA], BF16, tag="acts_bf")
            nc.vector.tensor_scalar_mul(acts_bf[:, 0:CQ], acts[:, 0:CQ], fq[:])
            nc.vector.tensor_scalar_mul(acts_bf[:, CQ:CQ + CKV],
                                        acts[:, CQ:CQ + CKV], fk[:])

            # k_pe rope (natural [s, 64] layout), cols 2048:2112
            kp0 = CQ + CKV
            kv1 = sbt.tile([S_SH, D_ROPE], F32, tag="kv1")
            kv2 = sbt.tile([S_SH, D_ROPE], F32, tag="kv2")
            nc.vector.tensor_mul(kv1[:], acts[:, kp0:kp0 + 64], cos_s_sb[:])
            nc.vector.tensor_mul(kv2[:, 0:32], acts[:, kp0 + 32:kp0 + 64], sin_sg_sb[:, 0:32])
            nc.vector.tensor_mul(kv2[:, 32:64], acts[:, kp0:kp0 + 32], sin_sg_sb[:, 32:64])
            nc.vector.tensor_add(acts_bf[:, kp0:kp0 + 64], kv1[:], kv2[:])

            # transpose all 17 chunks -> bounce [2112, 128] bf16
            bT = sba.tile([128, 17 * 128], BF16, tag="bT")
            for t in range(17):
                w = 128 if t < 16 else 64
                pt = ps_tp.tile([128, 512], BF16, tag="projbf")
                nc.tensor.transpose(pt[:w, 0:128], acts_bf[:, t * 128:t * 128 + w],
                                    ident[:])
                nc.scalar.copy(bT[:w, t * 128:(t + 1) * 128], pt[:w, 0:128])
                nc.sync.dma_start(agi[t * 128:t * 128 + w, :], bT[:w, t * 128:(t + 1) * 128])

        ps_proj = top.enter_context(tc.tile_pool(name="ps_proj", bufs=4, space="PSUM"))

        nc.gpsimd.collective_compute(
            "AllGather", OP.bypass,
            replica_groups=[list(range(N_CORES))],
            ins=[agi.opt()], outs=[ago.opt()],
        )

        # ================= Phase B: per-head-group projections + attention ==
        with ExitStack() as pb:
            ps_sc = pb.enter_context(tc.tile_pool(name="ps_sc", bufs=2, space="PSUM"))
            ps_ao = pb.enter_context(tc.tile_pool(name="ps_ao", bufs=2, space="PSUM"))
            sbg = pb.enter_context(tc.tile_pool(name="sbg", bufs=1))
            sbwq = pb.enter_context(tc.tile_pool(name="sbwq", bufs=2))
            sbh = pb.enter_context(tc.tile_pool(name="sbh", bufs=2))
            sbp = pb.enter_context(tc.tile_pool(name="sbp", bufs=2))
            sbv = pb.enter_context(tc.tile_pool(name="sbv", bufs=2))
            sbs = pb.enter_context(tc.tile_pool(name="sbs", bufs=3))

            def load_group_weights(g):
                h0 = g * G_HEADS
                qbnw = sbwq.tile([128, CQ // 128, G_HEADS * 128], BF16,
                                 name=f"qbnw{g}", tag="qbnw")
                qbpw = sbwq.tile([128, CQ // 128, G_HEADS * 64], BF16,
                                 name=f"qbpw{g}", tag="qbpw")
                kvbkw = sbwq.tile([128, CKV // 128, G_HEADS * 128], BF16,
                                  name=f"kvbkw{g}", tag="kvbkw")
                kvbvw = sbwq.tile([128, CKV // 128, G_HEADS * 128], BF16,
                                  name=f"kvbvw{g}", tag="kvbvw")
                for c in range(CQ // 128):
                    nc.sync.dma_start(qbnw[:, c, :], qbn.ap()[c * 128:(c + 1) * 128,
                                                              h0 * 128:(h0 + G_HEADS) * 128])
                    nc.sync.dma_start(qbpw[:, c, :], qbp.ap()[c * 128:(c + 1) * 128,
                                                              h0 * 64:(h0 + G_HEADS) * 64])
                for c in range(CKV // 128):
                    nc.sync.dma_start(kvbkw[:, c, :], kvbk.ap()[c * 128:(c + 1) * 128,
                                                                h0 * 128:(h0 + G_HEADS) * 128])
                    nc.sync.dma_start(kvbvw[:, c, :], kvbv.ap()[c * 128:(c + 1) * 128,
                                                                h0 * 128:(h0 + G_HEADS) * 128])
                return qbnw, qbpw, kvbkw, kvbvw

            wtiles = {0: load_group_weights(0)}

            # gathered activations, stitched per 512-wide s-tile
            qct = []
            ckv = []
            for st in range(2):
                q_t = sbg.tile([128, CQ // 128, 512], BF16, tag=f"qct{st}")
                k_t = sbg.tile([128, CKV // 128, 512], BF16, tag=f"ckv{st}")
                for r in range(4):
                    core = st * 4 + r
                    base = core * CA
                    for c in range(CQ // 128):
                        nc.sync.dma_start(
                            q_t[:, c, r * 128:(r + 1) * 128],
                            ago[base + c * 128:base + (c + 1) * 128, :])
                    for c in range(CKV // 128):
                        nc.sync.dma_start(
                            k_t[:, c, r * 128:(r + 1) * 128],
                            ago[base + CQ + c * 128:base + CQ + (c + 1) * 128, :])
                qct.append(q_t)
                ckv.append(k_t)
            kpe2 = sbg.tile([128, S], BF16, tag="kpe2")
            for core in range(N_CORES):
                base = core * CA + CQ + CKV
                nc.sync.dma_start(kpe2[0:64, core * 128:(core + 1) * 128],
                                  ago[base:base + 64, :])
                nc.sync.dma_start(kpe2[64:128, core * 128:(core + 1) * 128],
                                  ago[base:base + 64, :])

            for g in range(N_GROUPS):
                h0 = g * G_HEADS
                if g + 1 < N_GROUPS:
                    wtiles[g + 1] = load_group_weights(g + 1)
                qbnw, qbpw, kvbkw, kvbvw = wtiles.pop(g)

                # --- qT_nope / kT_nope per head; qT_pe pair; v ---
                qTn = []
                kTn = []
                for i in range(G_HEADS):
                    qt_t = sbh.tile([128, S], BF16, tag="qTn")
                    ps2 = [ps_proj.tile([128, 512], F32, name=f"ps2_{st}", tag="proj")
                           for st in range(2)]
                    for c in range(CQ // 128):
                        for st in range(2):
                            nc.tensor.matmul(ps2[st][:],
                                             qbnw[:, c, i * 128:(i + 1) * 128],
                                             qct[st][:, c, :],
                                             start=(c == 0), stop=(c == CQ // 128 - 1))
                    for st in range(2):
                        nc.scalar.copy(qt_t[:, st * 512:(st + 1) * 512], ps2[st][:])
                    qTn.append(qt_t)
                    kt_t = sbh.tile([128, S], BF16, tag="kTn")
                    ps2 = [ps_proj.tile([128, 512], F32, name=f"ps2_{st}", tag="proj")
                           for st in range(2)]
                    for c in range(CKV // 128):
                        for st in range(2):
                            nc.tensor.matmul(ps2[st][:],
                                             kvbkw[:, c, i * 128:(i + 1) * 128],
                                             ckv[st][:, c, :],
                                             start=(c == 0), stop=(c == CKV // 128 - 1))
                    for st in range(2):
                        nc.scalar.copy(kt_t[:, st * 512:(st + 1) * 512], ps2[st][:])
                    kTn.append(kt_t)

                qp_raw = sbp.tile([128, S], BF16, tag="qp_raw")
                ps2 = [ps_proj.tile([128, 512], F32, name=f"ps2_{st}", tag="proj")
                       for st in range(2)]
                for c in range(CQ // 128):
                    for st in range(2):
                        nc.tensor.matmul(ps2[st][:], qbpw[:, c, :], qct[st][:, c, :],
                                         start=(c == 0), stop=(c == CQ // 128 - 1))
                for st in range(2):
                    nc.scalar.copy(qp_raw[:, st * 512:(st + 1) * 512], ps2[st][:])
                # rope on the head-pair tile: rows [0:64]=head h0, [64:128]=h0+1
                qTp = sbh.tile([128, S], BF16, tag="qTp")
                rm = sbp.tile([128, S], BF16, tag="ropem")
                rs = sbp.tile([128, S], BF16, tag="ropes")
                nc.vector.tensor_mul(rm[:], qp_raw[:], cos2t_sb[:])
                # rs = swap32(qp_raw), then multiply by the sign-baked sin table
                for b in range(4):
                    r0 = b * 32
                    r1 = r0 + 32 if b % 2 == 0 else r0 - 32
                    nc.vector.tensor_copy(rs[r0:r0 + 32, :], qp_raw[r1:r1 + 32, :])
                nc.vector.tensor_mul(rs[:], rs[:], sin2tg_sb[:])
                nc.vector.tensor_add(qTp[:], rm[:], rs[:])

                v_g = sbv.tile([128, 8, G_HEADS * 128], BF16, tag="v_g")
                for sc in range(8):
                    st = sc // 4
                    psum = ps_proj.tile([128, 512], F32, tag="proj")
                    nn = G_HEADS * 128
                    for c in range(CKV // 128):
                        nc.tensor.matmul(
                            psum[:, :nn],
                            ckv[st][:, c, (sc % 4) * 128:(sc % 4 + 1) * 128],
                            kvbvw[:, c, :],
                            start=(c == 0), stop=(c == CKV // 128 - 1))
                    nc.vector.tensor_copy(v_g[:, sc, :], psum[:, :nn])

                # --- attention for each head in the group ---
                for i in range(G_HEADS):
                    h_loc = h0 + i
                    for qt in range(QT):
                        kmax = 4 * (qt + 1)
                        psum_o = ps_ao.tile([128, 512], F32, tag="o")
                        sums = sbs.tile([128, 512], F32R, tag="sums")
                        for kc in range(kmax):
                            ps = ps_sc.tile([128, 512], F32, tag="s")
                            nc.tensor.matmul(ps[:], kTn[i][:, kc * 128:(kc + 1) * 128],
                                             qTn[i][:, qt * 512:(qt + 1) * 512],
                                             start=True, stop=False)
                            b = i * 64
                            nc.tensor.matmul(ps[:], kpe2[b:b + 64, kc * 128:(kc + 1) * 128],
                                             qTp[b:b + 64, qt * 512:(qt + 1) * 512],
                                             start=False, stop=True)
                            pt = sbs.tile([128, 512], BF16, tag="pt")
                            nc.scalar.activation(pt[:], ps[:], AF.Exp, scale=SCALE)
                            if kc >= 4 * qt:
                                nc.vector.tensor_mul(pt[:], pt[:],
                                                     masks_sb[:, kc - 4 * qt, :])
                            if kc == 0:
                                nc.vector.tensor_copy(sums[:], pt[:])
                            else:
                                nc.vector.tensor_add(sums[:], sums[:], pt[:])
                            nc.tensor.matmul(psum_o[:],
                                             v_g[:, kc, i * 128:(i + 1) * 128], pt[:],
                                             start=(kc == 0), stop=(kc == kmax - 1))
                        pss = ps_sc.tile([128, 512], F32, tag="s")
                        nc.tensor.matmul(pss[0:1, :], ones_col_sb[:], sums[:],
                                         start=True, stop=True)
                        rec = sbs.tile([1, 512], F32R, tag="rec")
                        _scalar_reciprocal(nc, rec[:], pss[0:1, :])
                        psb = ps_sc.tile([128, 512], F32, tag="s")
                        nc.tensor.matmul(psb[:], ones_row_sb[:], rec[:],
                                         start=True, stop=True)
                        bsb = sbs.tile([128, 512], F32, tag="bsb")
                        nc.scalar.copy(bsb[:], psb[:])
                        nc.vector.tensor_mul(outs_all[:, h_loc, qt * 512:(qt + 1) * 512],
                                             psum_o[:], bsb[:])

        # ================= Phase C: partial output projection ===============
        with ExitStack() as pc:
            sbow = pc.enter_context(tc.tile_pool(name="sbow", bufs=2))
            sbos = pc.enter_context(tc.tile_pool(name="sbos", bufs=3))
            for nt in range(HID // 512):
                owt = sbow.tile([128, HG, 512], BF16, tag="ow")
                for hc in range(HG):
                    nc.sync.dma_start(owt[:, hc, :],
                                      ow.ap()[hc * 128:(hc + 1) * 128,
                                              nt * 512:(nt + 1) * 512])
                for st in range(8):
                    psum = ps_proj.tile([128, 512], F32, tag="proj")
                    for hc in range(HG):
                        nc.tensor.matmul(psum[:],
                                         outs_all[:, hc, st * 128:(st + 1) * 128],
                                         owt[:, hc, :],
                                         start=(hc == 0), stop=(hc == HG - 1))
                    osb = sbos.tile([128, 512], F32, tag="osb")
                    nc.scalar.copy(osb[:], psum[:])
                    nc.sync.dma_start(out.ap()[st * 128:(st + 1) * 128,
                                               nt * 512:(nt + 1) * 512], osb[:])

    nc.compile()
    return nc


def _host_inputs(hidden_states, position_ids, q_a_weight, q_a_layernorm_weight,
                 q_b_weight, kv_a_weight, kv_a_layernorm_weight, kv_b_weight,
                 o_weight):
    bf = ml_dtypes.bfloat16
    x = np.asarray(hidden_states, np.float32).reshape(S, HID)
    pos = np.asarray(position_ids, np.float64).reshape(S)
    q_a_w = np.asarray(q_a_weight, np.float32)
    q_ln = np.asarray(q_a_layernorm_weight, np.float32)
    q_b_w = np.asarray(q_b_weight, np.float32)
    kv_a_w = np.asarray(kv_a_weight, np.float32)
    kv_ln = np.asarray(kv_a_layernorm_weight, np.float32)
    kv_b_w = np.asarray(kv_b_weight, np.float32)
    o_w = np.asarray(o_weight, np.float32)

    wa = np.concatenate([q_a_w, kv_a_w], axis=1).astype(bf)    # [HID, 2112]
    xT = np.ascontiguousarray(x.T).astype(bf)                   # [HID, S]

    # fold the rms-norm weights into the b-projections
    qb = (q_ln[:, None] * q_b_w).reshape(CQ, H, D_Q)
    kvb = (kv_ln[:, None] * kv_b_w).reshape(CKV, H, D_NOPE + D_V)

    # rope tables
    inv_freq = 1.0 / (10000.0 ** (np.arange(0, D_ROPE, 2, dtype=np.float64) / D_ROPE))
    freqs = pos[:, None] * inv_freq[None, :]                # [S, 32]
    emb = np.concatenate([freqs, freqs], axis=-1)           # [S, 64]
    cos = np.cos(emb).astype(np.float32)
    sin = np.sin(emb).astype(np.float32)
    sin_sg = np.concatenate([-sin[:, :32], sin[:, 32:]], axis=1)  # [S, 64]
    cosT = np.ascontiguousarray(cos.T)                      # [64, S]
    sinT_sg = np.ascontiguousarray(sin_sg.T)                # [64, S]
    cos2t = np.concatenate([cosT, cosT], axis=0).astype(bf)     # [128, S]
    sin2tg = np.concatenate([sinT_sg, sinT_sg], axis=0).astype(bf)  # [128, S]

    # causal masks for the 4 diagonal offsets
    masks = np.zeros((4, 128, 512), np.float32)
    i = np.arange(128)[:, None]
    j = np.arange(512)[None, :]
    for m in range(4):
        masks[m] = ((i + m * 128) <= j).astype(np.float32)
    masks = masks.reshape(512, 512).astype(bf)

    ones_col = np.ones((128, 1), np.float32)
    ones_row = np.ones((1, 128), np.float32)

    in_maps = []
    for c in range(N_CORES):
        hs = slice(c * HG, (c + 1) * HG)
        in_maps.append({
            "xT": np.ascontiguousarray(xT[:, c * S_SH:(c + 1) * S_SH]),
            "wa": wa,
            "qbn": np.ascontiguousarray(
                qb[:, hs, :D_NOPE].reshape(CQ, HG * D_NOPE)).astype(bf),
            "qbp": np.ascontiguousarray(
                qb[:, hs, D_NOPE:].reshape(CQ, HG * D_ROPE)).astype(bf),
            "kvbk": np.ascontiguousarray(
                kvb[:, hs, :D_NOPE].reshape(CKV, HG * D_NOPE)).astype(bf),
            "kvbv": np.ascontiguousarray(
                kvb[:, hs, D_NOPE:].reshape(CKV, HG * D_V)).astype(bf),
            "ow": np.ascontiguousarray(
                o_w[c * HG * D_V:(c + 1) * HG * D_V, :]).astype(bf),
            "cos_s": np.ascontiguousarray(cos[c * S_SH:(c + 1) * S_SH, :]),
            "sin_sg": np.ascontiguousarray(sin_sg[c * S_SH:(c + 1) * S_SH, :]),
            "cos2t": cos2t,
            "sin2tg": sin2tg,
            "masks": masks,
            "ones_col": ones_col,
            "ones_row": ones_row,
        })
    return in_maps


def kernel(**inputs):
    global LAST_EXEC_NS
    trace = bool(inputs.pop("_trace", False))
    in_maps = _host_inputs(**inputs)
    if "nc" not in _CACHE:
        _CACHE["nc"] = _build_nc()
    nc = _CACHE["nc"]
    res = bass_utils.run_bass_kernel_spmd(
        nc, in_maps, core_ids=list(range(N_CORES)), trace=trace)
    LAST_EXEC_NS = res.exec_time_ns
    total = np.zeros((S, HID), np.float64)
    for c in range(N_CORES):
        total += res.results[c]["out"].astype(np.float64)
    return total.astype(np.float32).reshape(1, 1, S, HID)


# revision 11
# speedup vs baseline: 2.2665x; 1.0085x over previous
"""DeepseekV3 MLA attention prefill (S=1024, H=128 heads, HID=7168) on 8 TRN2
NeuronCores.

Sharding: tensor-parallel over heads (16 heads/core) for q_b/kv_b/attention/
o_proj; the low-rank input projections (q_a / kv_a) are sequence-sharded
(128 rows/core) and exchanged with one small AllGather of the transposed,
rms-normed activations. Each core emits a partial output projection
(contraction over its own 16 heads); the host sums the 8 partials.

All matmuls run in bf16 (fp32r draws enough PE power to trip EDPP duty
throttling to 50%; bf16 runs at the same 1 cycle/row without it and halves
weight DMA + LDWEIGHTS traffic). PSUM accumulation and softmax statistics
stay float32.
"""
import math
import numpy as np
import ml_dtypes

import concourse.bass as bass
import concourse.mybir as mybir
import concourse.bacc as bacc
import concourse.tile as tile
import concourse.bass_utils as bass_utils
from concourse.masks import make_identity
from contextlib import ExitStack

F32 = mybir.dt.float32
F32R = mybir.dt.float32r
BF16 = mybir.dt.bfloat16
AF = mybir.ActivationFunctionType
OP = mybir.AluOpType

N_CORES = 8
S = 1024
HID = 7168
H = 128
HG = H // N_CORES          # 16 heads per core
D_NOPE = 128
D_ROPE = 64
D_Q = D_NOPE + D_ROPE      # 192
D_V = 128
CQ = 1536                  # q lora rank
CKV = 512                  # kv lora rank
CA = CQ + CKV + D_ROPE     # 2112 fused a-proj cols
S_SH = S // N_CORES        # 128 sequence rows per core
CC_A = HID // 128          # 56 contraction chunks for a-proj
NT_A = [(0, 512), (512, 512), (1024, 512), (1536, 512), (2048, 64)]
SCALE = 1.0 / math.sqrt(D_Q)
EPS = 1e-6
G_HEADS = 2                # heads per group
N_GROUPS = HG // G_HEADS   # 8 groups
QT = 2                     # q-tiles of 512 per head
LAST_EXEC_NS = None

_CACHE = {}


def _build_nc():
    nc = bacc.Bacc("TRN2", target_bir_lowering=False, debug=False,
                   num_devices=N_CORES)

    xT = nc.dram_tensor("xT", [HID, S_SH], BF16, kind="ExternalInput")
    wa = nc.dram_tensor("wa", [HID, CA], BF16, kind="ExternalInput")
    qbn = nc.dram_tensor("qbn", [CQ, HG * D_NOPE], BF16, kind="ExternalInput")
    qbp = nc.dram_tensor("qbp", [CQ, HG * D_ROPE], BF16, kind="ExternalInput")
    kvbk = nc.dram_tensor("kvbk", [CKV, HG * D_NOPE], BF16, kind="ExternalInput")
    kvbv = nc.dram_tensor("kvbv", [CKV, HG * D_V], BF16, kind="ExternalInput")
    ow = nc.dram_tensor("ow", [HG * D_V, HID], BF16, kind="ExternalInput")
    cos_s = nc.dram_tensor("cos_s", [S_SH, D_ROPE], F32, kind="ExternalInput")
    sin_sg = nc.dram_tensor("sin_sg", [S_SH, D_ROPE], F32, kind="ExternalInput")
    cos2t = nc.dram_tensor("cos2t", [128, S], BF16, kind="ExternalInput")
    sin2tg = nc.dram_tensor("sin2tg", [128, S], BF16, kind="ExternalInput")
    masks = nc.dram_tensor("masks", [512, 512], BF16, kind="ExternalInput")
    ones_col = nc.dram_tensor("ones_col", [128, 1], F32R, kind="ExternalInput")
    ones_row = nc.dram_tensor("ones_row", [1, 128], F32R, kind="ExternalInput")
    out = nc.dram_tensor("out", [S, HID], F32, kind="ExternalOutput")

    with tile.TileContext(nc) as tc, ExitStack() as top:
        const = top.enter_context(tc.tile_pool(name="const", bufs=1))
        outsp = top.enter_context(tc.tile_pool(name="outsp", bufs=1))
        dram = top.enter_context(tc.tile_pool(name="dram", bufs=1, space="DRAM"))
        # ---- constants in SBUF ----
        ident = const.tile([128, 128], BF16, tag="ident")
        make_identity(nc, ident[:])
        masks_sb = const.tile([128, 4, 512], BF16, tag="masks")
        for m in range(4):
            nc.sync.dma_start(masks_sb[:, m, :], masks.ap()[m * 128:(m + 1) * 128, :])
        cos_s_sb = const.tile([S_SH, D_ROPE], F32, tag="coss")
        sin_sg_sb = const.tile([S_SH, D_ROPE], F32, tag="sinsg")
        nc.sync.dma_start(cos_s_sb[:], cos_s.ap())
        nc.sync.dma_start(sin_sg_sb[:], sin_sg.ap())
        cos2t_sb = const.tile([128, S], BF16, tag="cos2t")
        sin2tg_sb = const.tile([128, S], BF16, tag="sin2tg")
        nc.sync.dma_start(cos2t_sb[:], cos2t.ap())
        nc.sync.dma_start(sin2tg_sb[:], sin2tg.ap())
        ones_col_sb = const.tile([128, 1], F32R, tag="onesc")
        ones_row_sb = const.tile([1, 128], F32R, tag="onesr")
        nc.sync.dma_start(ones_col_sb[:], ones_col.ap())
        nc.sync.dma_start(ones_row_sb[:], ones_row.ap())

        # attention outputs for all 16 local heads, [dv, head, s], bf16
        outs_all = outsp.tile([128, HG, S], BF16, tag="outs_all")

        CKP = CKV + D_ROPE  # 576 ckv+kpe rows
        agi_q = dram.tile([CQ, S_SH], BF16, tag="agi_q")
        ago_q = dram.tile([CQ * N_CORES, S_SH], BF16, tag="ago_q")
        agi_kv = dram.tile([CKP, S_SH], BF16, tag="agi_kv")
        ago_kv = dram.tile([CKP * N_CORES, S_SH], BF16, tag="ago_kv")

        # ================= Phase A: fused a-proj + rmsnorm + kpe rope ======
        with ExitStack() as pa:
            sba = pa.enter_context(tc.tile_pool(name="sba", bufs=1))
            sbw = pa.enter_context(tc.tile_pool(name="sbw", bufs=3))
            sbt = pa.enter_context(tc.tile_pool(name="sbt", bufs=2))
            ps_a = pa.enter_context(tc.tile_pool(name="ps_a", bufs=1, space="PSUM"))
            ps_tp = pa.enter_context(tc.tile_pool(name="ps_tp", bufs=2, space="PSUM"))

            xT_sb = sba.tile([128, CC_A, S_SH], BF16, tag="xT")
            for cc in range(CC_A):
                nc.sync.dma_start(xT_sb[:, cc, :], xT.ap()[cc * 128:(cc + 1) * 128, :])
            # 5 live psum banks accumulate the full [128, 2112] activation row
            pa_t = [ps_a.tile([128, 512], F32, name=f"pa{nt}", tag=f"pa{nt}")
                    for nt in range(5)]
            for cc in range(CC_A):
                wt = sbw.tile([128, CA], BF16, tag="wa")
                nc.sync.dma_start(wt[:], wa.ap()[cc * 128:(cc + 1) * 128, :])
                for nt, (d0, dn) in enumerate(NT_A):
                    nc.tensor.matmul(pa_t[nt][:, :dn], xT_sb[:, cc, :],
                                     wt[:, d0:d0 + dn],
                                     start=(cc == 0), stop=(cc == CC_A - 1))
            acts = sba.tile([S_SH, CA], F32, tag="acts")
            for nt, (d0, dn) in enumerate(NT_A):
                nc.scalar.copy(acts[:, d0:d0 + dn], pa_t[nt][:, :dn])

            # rmsnorm factors for qc (cols 0:1536) and ckv (cols 1536:2048)
            sq = sba.tile([S_SH, CQ + CKV], F32, tag="sq")
            nc.vector.tensor_mul(sq[:], acts[:, 0:CQ + CKV], acts[:, 0:CQ + CKV])
            fq = sbt.tile([S_SH, 1], F32, tag="fq")
            fk = sbt.tile([S_SH, 1], F32, tag="fk")
            nc.vector.reduce_sum(fq[:], sq[:, 0:CQ], axis=mybir.AxisListType.X)
            nc.vector.reduce_sum(fk[:], sq[:, CQ:CQ + CKV], axis=mybir.AxisListType.X)
            nc.vector.tensor_scalar(fq[:], fq[:], 1.0 / CQ, EPS, OP.mult, OP.add)
            nc.vector.tensor_scalar(fk[:], fk[:], 1.0 / CKV, EPS, OP.mult, OP.add)
            nc.vector.reciprocal(fq[:], fq[:])
            nc.vector.reciprocal(fk[:], fk[:])
            nc.scalar.activation(fq[:], fq[:], AF.Sqrt)
            nc.scalar.activation(fk[:], fk[:], AF.Sqrt)
            acts_bf = sba.tile([S_SH, CA], BF16, tag="acts_bf")
            nc.vector.tensor_scalar_mul(acts_bf[:, 0:CQ], acts[:, 0:CQ], fq[:])
            nc.vector.tensor_scalar_mul(acts_bf[:, CQ:CQ + CKV],
                                        acts[:, CQ:CQ + CKV], fk[:])

            # k_pe rope (natural [s, 64] layout), cols 2048:2112
            kp0 = CQ + CKV
            kv1 = sbt.tile([S_SH, D_ROPE], F32, tag="kv1")
            kv2 = sbt.tile([S_SH, D_ROPE], F32, tag="kv2")
            nc.vector.tensor_mul(kv1[:], acts[:, kp0:kp0 + 64], cos_s_sb[:])
            nc.vector.tensor_mul(kv2[:, 0:32], acts[:, kp0 + 32:kp0 + 64], sin_sg_sb[:, 0:32])
            nc.vector.tensor_mul(kv2[:, 32:64], acts[:, kp0:kp0 + 32], sin_sg_sb[:, 32:64])
            nc.vector.tensor_add(acts_bf[:, kp0:kp0 + 64], kv1[:], kv2[:])

            # transpose all 17 chunks -> bounce buffers (kv chunks first so
            # the kv AllGather can start early)
            bT = sba.tile([128, 17 * 128], BF16, tag="bT")
            for t in list(range(12, 17)) + list(range(12)):
                w = 128 if t < 16 else 64
                pt = ps_tp.tile([128, 512], BF16, tag="projbf")
                nc.tensor.transpose(pt[:w, 0:128], acts_bf[:, t * 128:t * 128 + w],
                                    ident[:])
                nc.scalar.copy(bT[:w, t * 128:(t + 1) * 128], pt[:w, 0:128])
                if t < 12:
                    nc.sync.dma_start(agi_q[t * 128:t * 128 + w, :],
                                      bT[:w, t * 128:(t + 1) * 128])
                else:
                    r0 = (t - 12) * 128
                    nc.sync.dma_start(agi_kv[r0:r0 + w, :],
                                      bT[:w, t * 128:(t + 1) * 128])

        ps_proj = top.enter_context(tc.tile_pool(name="ps_proj", bufs=3, space="PSUM"))

        nc.gpsimd.collective_compute(
            "AllGather", OP.bypass,
            replica_groups=[list(range(N_CORES))],
            ins=[agi_kv.opt()], outs=[ago_kv.opt()],
        )
        nc.gpsimd.collective_compute(
            "AllGather", OP.bypass,
            replica_groups=[list(range(N_CORES))],
            ins=[agi_q.opt()], outs=[ago_q.opt()],
        )

        # ================= Phase B: per-head-group projections + attention ==
        with ExitStack() as pb:
            ps_sc = pb.enter_context(tc.tile_pool(name="ps_sc", bufs=3, space="PSUM"))
            ps_ao = pb.enter_context(tc.tile_pool(name="ps_ao", bufs=2, space="PSUM"))
            sbg = pb.enter_context(tc.tile_pool(name="sbg", bufs=1))
            sbwq = pb.enter_context(tc.tile_pool(name="sbwq", bufs=2))
            sbh = pb.enter_context(tc.tile_pool(name="sbh", bufs=2))
            sbp = pb.enter_context(tc.tile_pool(name="sbp", bufs=2))
            sbv = pb.enter_context(tc.tile_pool(name="sbv", bufs=2))
            sbs = pb.enter_context(tc.tile_pool(name="sbs", bufs=3))

            def load_group_weights(g):
                h0 = g * G_HEADS
                qbnw = sbwq.tile([128, CQ // 128, G_HEADS * 128], BF16,
                                 name=f"qbnw{g}", tag="qbnw")
                qbpw = sbwq.tile([128, CQ // 128, G_HEADS * 64], BF16,
                                 name=f"qbpw{g}", tag="qbpw")
                kvbkw = sbwq.tile([128, CKV // 128, G_HEADS * 128], BF16,
                                  name=f"kvbkw{g}", tag="kvbkw")
                kvbvw = sbwq.tile([128, CKV // 128, G_HEADS * 128], BF16,
                                  name=f"kvbvw{g}", tag="kvbvw")
                for c in range(CQ // 128):
                    nc.sync.dma_start(qbnw[:, c, :], qbn.ap()[c * 128:(c + 1) * 128,
                                                              h0 * 128:(h0 + G_HEADS) * 128])
                    nc.sync.dma_start(qbpw[:, c, :], qbp.ap()[c * 128:(c + 1) * 128,
                                                              h0 * 64:(h0 + G_HEADS) * 64])
                for c in range(CKV // 128):
                    nc.sync.dma_start(kvbkw[:, c, :], kvbk.ap()[c * 128:(c + 1) * 128,
                                                                h0 * 128:(h0 + G_HEADS) * 128])
                    nc.sync.dma_start(kvbvw[:, c, :], kvbv.ap()[c * 128:(c + 1) * 128,
                                                                h0 * 128:(h0 + G_HEADS) * 128])
                return qbnw, qbpw, kvbkw, kvbvw

            wtiles = {0: load_group_weights(0)}

            # gathered activations, stitched per 512-wide s-tile
            qct = []
            ckv = []
            for st in range(2):
                q_t = sbg.tile([128, CQ // 128, 512], BF16, tag=f"qct{st}")
                k_t = sbg.tile([128, CKV // 128, 512], BF16, tag=f"ckv{st}")
                for r in range(4):
                    core = st * 4 + r
                    for c in range(CKV // 128):
                        nc.sync.dma_start(
                            k_t[:, c, r * 128:(r + 1) * 128],
                            ago_kv[core * CKP + c * 128:core * CKP + (c + 1) * 128, :])
                    for c in range(CQ // 128):
                        nc.sync.dma_start(
                            q_t[:, c, r * 128:(r + 1) * 128],
                            ago_q[core * CQ + c * 128:core * CQ + (c + 1) * 128, :])
                qct.append(q_t)
                ckv.append(k_t)
            srows = sbg.tile([HG * QT, 512], F32, tag="srows")
            rec_all = sbg.tile([HG * QT, 512], F32R, tag="rec_all")
            kpe2 = sbg.tile([128, S], BF16, tag="kpe2")
            for core in range(N_CORES):
                base = core * CKP + CKV
                nc.sync.dma_start(kpe2[0:64, core * 128:(core + 1) * 128],
                                  ago_kv[base:base + 64, :])
                nc.sync.dma_start(kpe2[64:128, core * 128:(core + 1) * 128],
                                  ago_kv[base:base + 64, :])

            for g in range(N_GROUPS):
                h0 = g * G_HEADS
                if g + 1 < N_GROUPS:
                    wtiles[g + 1] = load_group_weights(g + 1)
                qbnw, qbpw, kvbkw, kvbvw = wtiles.pop(g)

                # --- kv-side first (only needs the kv AllGather) ---
                kTn = []
                for i in range(G_HEADS):
                    kt_t = sbh.tile([128, S], BF16, tag="kTn")
                    ps2 = [ps_proj.tile([128, 512], F32, name=f"ps2_{st}", tag="proj")
                           for st in range(2)]
                    for c in range(CKV // 128):
                        for st in range(2):
                            nc.tensor.matmul(ps2[st][:],
                                             kvbkw[:, c, i * 128:(i + 1) * 128],
                                             ckv[st][:, c, :],
                                             start=(c == 0), stop=(c == CKV // 128 - 1))
                    for st in range(2):
                        nc.scalar.copy(kt_t[:, st * 512:(st + 1) * 512], ps2[st][:])
                    kTn.append(kt_t)

                v_g = sbv.tile([128, 8, G_HEADS * 128], BF16, tag="v_g")
                for sc in range(8):
                    st = sc // 4
                    psum = ps_proj.tile([128, 512], F32, tag="proj")
                    nn = G_HEADS * 128
                    for c in range(CKV // 128):
                        nc.tensor.matmul(
                            psum[:, :nn],
                            ckv[st][:, c, (sc % 4) * 128:(sc % 4 + 1) * 128],
                            kvbvw[:, c, :],
                            start=(c == 0), stop=(c == CKV // 128 - 1))
                    nc.vector.tensor_copy(v_g[:, sc, :], psum[:, :nn])

                # --- q-side (needs the q AllGather) ---
                qTn = []
                for i in range(G_HEADS):
                    qt_t = sbh.tile([128, S], BF16, tag="qTn")
                    ps2 = [ps_proj.tile([128, 512], F32, name=f"ps2_{st}", tag="proj")
                           for st in range(2)]
                    for c in range(CQ // 128):
                        for st in range(2):
                            nc.tensor.matmul(ps2[st][:],
                                             qbnw[:, c, i * 128:(i + 1) * 128],
                                             qct[st][:, c, :],
                                             start=(c == 0), stop=(c == CQ // 128 - 1))
                    for st in range(2):
                        nc.scalar.copy(qt_t[:, st * 512:(st + 1) * 512], ps2[st][:])
                    qTn.append(qt_t)

                qp_raw = sbp.tile([128, S], BF16, tag="qp_raw")
                ps2 = [ps_proj.tile([128, 512], F32, name=f"ps2_{st}", tag="proj")
                       for st in range(2)]
                for c in range(CQ // 128):
                    for st in range(2):
                        nc.tensor.matmul(ps2[st][:], qbpw[:, c, :], qct[st][:, c, :],
                                         start=(c == 0), stop=(c == CQ // 128 - 1))
                for st in range(2):
                    nc.scalar.copy(qp_raw[:, st * 512:(st + 1) * 512], ps2[st][:])
                # rope on the head-pair tile: rows [0:64]=head h0, [64:128]=h0+1
                qTp = sbh.tile([128, S], BF16, tag="qTp")
                rm = sbp.tile([128, S], BF16, tag="ropem")
                rs = sbp.tile([128, S], BF16, tag="ropes")
                nc.vector.tensor_mul(rm[:], qp_raw[:], cos2t_sb[:])
                # rs = swap32(qp_raw), then multiply by the sign-baked sin table
                for b in range(4):
                    r0 = b * 32
                    r1 = r0 + 32 if b % 2 == 0 else r0 - 32
                    nc.vector.tensor_copy(rs[r0:r0 + 32, :], qp_raw[r1:r1 + 32, :])
                nc.vector.tensor_mul(rs[:], rs[:], sin2tg_sb[:])
                nc.vector.tensor_add(qTp[:], rm[:], rs[:])

                # --- attention for each head in the group ---
                for i in range(G_HEADS):
                    h_loc = h0 + i
                    for qt in range(QT):
                        kmax = 4 * (qt + 1)
                        psum_o = ps_ao.tile([128, 512], F32, tag="o")
                        sums = sbs.tile([128, 512], F32R, tag="sums")
                        for kc in range(kmax):
                            ps = ps_sc.tile([128, 512], F32, tag="s")
                            nc.tensor.matmul(ps[:], kTn[i][:, kc * 128:(kc + 1) * 128],
                                             qTn[i][:, qt * 512:(qt + 1) * 512],
                                             start=True, stop=False)
                            b = i * 64
                            nc.tensor.matmul(ps[:], kpe2[b:b + 64, kc * 128:(kc + 1) * 128],
                                             qTp[b:b + 64, qt * 512:(qt + 1) * 512],
                                             start=False, stop=True)
                            pt = sbs.tile([128, 512], BF16, tag="pt")
                            nc.scalar.activation(pt[:], ps[:], AF.Exp, scale=SCALE)
                            if kc >= 4 * qt:
                                nc.vector.tensor_mul(pt[:], pt[:],
                                                     masks_sb[:, kc - 4 * qt, :])
                            if kc == 0:
                                nc.vector.tensor_copy(sums[:], pt[:])
                            else:
                                nc.vector.tensor_add(sums[:], sums[:], pt[:])
                            nc.tensor.matmul(psum_o[:],
                                             v_g[:, kc, i * 128:(i + 1) * 128], pt[:],
                                             start=(kc == 0), stop=(kc == kmax - 1))
                        pss = ps_sc.tile([128, 512], F32, tag="s")
                        nc.tensor.matmul(pss[0:1, :], ones_col_sb[:], sums[:],
                                         start=True, stop=True)
                        row = h_loc * QT + qt
                        srow = sbs.tile([1, 512], F32, tag="srow")
                        nc.scalar.copy(srow[:], pss[0:1, :])
                        nc.sync.dma_start(srows[row:row + 1, :], srow[:])
                        nc.scalar.copy(outs_all[:, h_loc, qt * 512:(qt + 1) * 512],
                                       psum_o[:])

            # --- deferred softmax normalization: one batched reciprocal ---
            with nc.allow_low_precision(reason="softmax recip in f32r"):
                nc.vector.reciprocal(rec_all[:], srows[:])
            for h_loc in range(HG):
                for qt in range(QT):
                    row = h_loc * QT + qt
                    rrow = sbs.tile([1, 512], F32R, tag="rrow")
                    nc.sync.dma_start(rrow[:], rec_all[row:row + 1, :])
                    psb = ps_sc.tile([128, 512], F32, tag="s")
                    nc.tensor.matmul(psb[:], ones_row_sb[:], rrow[:],
                                     start=True, stop=True)
                    bsb = sbs.tile([128, 512], BF16, tag="bsb")
                    nc.scalar.copy(bsb[:], psb[:])
                    nc.vector.tensor_mul(
                        outs_all[:, h_loc, qt * 512:(qt + 1) * 512],
                        outs_all[:, h_loc, qt * 512:(qt + 1) * 512], bsb[:])

        # ================= Phase C: partial output projection ===============
        with ExitStack() as pc:
            sbow = pc.enter_context(tc.tile_pool(name="sbow", bufs=2))
            sbos = pc.enter_context(tc.tile_pool(name="sbos", bufs=3))
            for nt in range(HID // 512):
                owt = sbow.tile([128, HG, 512], BF16, tag="ow")
                for hc in range(HG):
                    nc.sync.dma_start(owt[:, hc, :],
                                      ow.ap()[hc * 128:(hc + 1) * 128,
                                              nt * 512:(nt + 1) * 512])
                for st in range(8):
                    psum = ps_proj.tile([128, 512], F32, tag="proj")
                    for hc in range(HG):
                        nc.tensor.matmul(psum[:],
                                         outs_all[:, hc, st * 128:(st + 1) * 128],
                                         owt[:, hc, :],
                                         start=(hc == 0), stop=(hc == HG - 1))
                    osb = sbos.tile([128, 512], F32, tag="osb")
                    nc.scalar.copy(osb[:], psum[:])
                    nc.sync.dma_start(out.ap()[st * 128:(st + 1) * 128,
                                               nt * 512:(nt + 1) * 512], osb[:])

    nc.compile()
    return nc


def _host_inputs(hidden_states, position_ids, q_a_weight, q_a_layernorm_weight,
                 q_b_weight, kv_a_weight, kv_a_layernorm_weight, kv_b_weight,
                 o_weight):
    bf = ml_dtypes.bfloat16
    x = np.asarray(hidden_states, np.float32).reshape(S, HID)
    pos = np.asarray(position_ids, np.float64).reshape(S)
    q_a_w = np.asarray(q_a_weight, np.float32)
    q_ln = np.asarray(q_a_layernorm_weight, np.float32)
    q_b_w = np.asarray(q_b_weight, np.float32)
    kv_a_w = np.asarray(kv_a_weight, np.float32)
    kv_ln = np.asarray(kv_a_layernorm_weight, np.float32)
    kv_b_w = np.asarray(kv_b_weight, np.float32)
    o_w = np.asarray(o_weight, np.float32)

    wa = np.concatenate([q_a_w, kv_a_w], axis=1).astype(bf)    # [HID, 2112]
    xT = np.ascontiguousarray(x.T).astype(bf)                   # [HID, S]

    # fold the rms-norm weights into the b-projections
    qb = (q_ln[:, None] * q_b_w).reshape(CQ, H, D_Q)
    kvb = (kv_ln[:, None] * kv_b_w).reshape(CKV, H, D_NOPE + D_V)

    # rope tables
    inv_freq = 1.0 / (10000.0 ** (np.arange(0, D_ROPE, 2, dtype=np.float64) / D_ROPE))
    freqs = pos[:, None] * inv_freq[None, :]                # [S, 32]
    emb = np.concatenate([freqs, freqs], axis=-1)           # [S, 64]
    cos = np.cos(emb).astype(np.float32)
    sin = np.sin(emb).astype(np.float32)
    sin_sg = np.concatenate([-sin[:, :32], sin[:, 32:]], axis=1)  # [S, 64]
    cosT = np.ascontiguousarray(cos.T)                      # [64, S]
    sinT_sg = np.ascontiguousarray(sin_sg.T)                # [64, S]
    cos2t = np.concatenate([cosT, cosT], axis=0).astype(bf)     # [128, S]
    sin2tg = np.concatenate([sinT_sg, sinT_sg], axis=0).astype(bf)  # [128, S]

    # causal masks for the 4 diagonal offsets
    masks = np.zeros((4, 128, 512), np.float32)
    i = np.arange(128)[:, None]
    j = np.arange(512)[None, :]
    for m in range(4):
        masks[m] = ((i + m * 128) <= j).astype(np.float32)
    masks = masks.reshape(512, 512).astype(bf)

    ones_col = np.ones((128, 1), np.float32)
    ones_row = np.ones((1, 128), np.float32)

    in_maps = []
    for c in range(N_CORES):
        hs = slice(c * HG, (c + 1) * HG)
        in_maps.append({
            "xT": np.ascontiguousarray(xT[:, c * S_SH:(c + 1) * S_SH]),
            "wa": wa,
            "qbn": np.ascontiguousarray(
                qb[:, hs, :D_NOPE].reshape(CQ, HG * D_NOPE)).astype(bf),
            "qbp": np.ascontiguousarray(
                qb[:, hs, D_NOPE:].reshape(CQ, HG * D_ROPE)).astype(bf),
            "kvbk": np.ascontiguousarray(
                kvb[:, hs, :D_NOPE].reshape(CKV, HG * D_NOPE)).astype(bf),
            "kvbv": np.ascontiguousarray(
                kvb[:, hs, D_NOPE:].reshape(CKV, HG * D_V)).astype(bf),
            "ow": np.ascontiguousarray(
                o_w[c * HG * D_V:(c + 1) * HG * D_V, :]).astype(bf),
            "cos_s": np.ascontiguousarray(cos[c * S_SH:(c + 1) * S_SH, :]),
            "sin_sg": np.ascontiguousarray(sin_sg[c * S_SH:(c + 1) * S_SH, :]),
            "cos2t": cos2t,
            "sin2tg": sin2tg,
            "masks": masks,
            "ones_col": ones_col,
            "ones_row": ones_row,
        })
    return in_maps


def kernel(**inputs):
    global LAST_EXEC_NS
    trace = bool(inputs.pop("_trace", False))
    in_maps = _host_inputs(**inputs)
    if "nc" not in _CACHE:
        _CACHE["nc"] = _build_nc()
    nc = _CACHE["nc"]
    res = bass_utils.run_bass_kernel_spmd(
        nc, in_maps, core_ids=list(range(N_CORES)), trace=trace)
    LAST_EXEC_NS = res.exec_time_ns
    total = np.zeros((S, HID), np.float64)
    for c in range(N_CORES):
        total += res.results[c]["out"].astype(np.float64)
    return total.astype(np.float32).reshape(1, 1, S, HID)


# revision 13
# speedup vs baseline: 2.3372x; 1.0312x over previous
"""DeepseekV3 MLA attention prefill (S=1024, H=128 heads, HID=7168) on 8 TRN2
NeuronCores.

Sharding: tensor-parallel over heads (16 heads/core) for q_b/kv_b/attention/
o_proj; the low-rank input projections (q_a / kv_a) are sequence-sharded
(128 rows/core) and exchanged with one small AllGather of the transposed,
rms-normed activations. Each core emits a partial output projection
(contraction over its own 16 heads); the host sums the 8 partials.

All matmuls run in bf16 (fp32r draws enough PE power to trip EDPP duty
throttling to 50%; bf16 runs at the same 1 cycle/row without it and halves
weight DMA + LDWEIGHTS traffic). PSUM accumulation and softmax statistics
stay float32.
"""
import math
import numpy as np
import ml_dtypes

import concourse.bass as bass
import concourse.mybir as mybir
import concourse.bacc as bacc
import concourse.tile as tile
import concourse.bass_utils as bass_utils
from concourse.masks import make_identity
from contextlib import ExitStack

F32 = mybir.dt.float32
F32R = mybir.dt.float32r
BF16 = mybir.dt.bfloat16
AF = mybir.ActivationFunctionType
OP = mybir.AluOpType

N_CORES = 8
S = 1024
HID = 7168
H = 128
HG = H // N_CORES          # 16 heads per core
D_NOPE = 128
D_ROPE = 64
D_Q = D_NOPE + D_ROPE      # 192
D_V = 128
CQ = 1536                  # q lora rank
CKV = 512                  # kv lora rank
CA = CQ + CKV + D_ROPE     # 2112 fused a-proj cols
S_SH = S // N_CORES        # 128 sequence rows per core
CC_A = HID // 128          # 56 contraction chunks for a-proj
NT_A = [(0, 512), (512, 512), (1024, 512), (1536, 512), (2048, 64)]
SCALE = 1.0 / math.sqrt(D_Q)
EPS = 1e-6
G_HEADS = 2                # heads per group
N_GROUPS = HG // G_HEADS   # 8 groups
QT = 2                     # q-tiles of 512 per head
LAST_EXEC_NS = None

_CACHE = {}


def _build_nc():
    nc = bacc.Bacc("TRN2", target_bir_lowering=False, debug=False,
                   num_devices=N_CORES)

    xT = nc.dram_tensor("xT", [HID, S_SH], BF16, kind="ExternalInput")
    wa = nc.dram_tensor("wa", [HID, CA], BF16, kind="ExternalInput")
    qbn = nc.dram_tensor("qbn", [CQ, HG * D_NOPE], BF16, kind="ExternalInput")
    qbp = nc.dram_tensor("qbp", [CQ, HG * D_ROPE], BF16, kind="ExternalInput")
    kvbk = nc.dram_tensor("kvbk", [CKV, HG * D_NOPE], BF16, kind="ExternalInput")
    kvbv = nc.dram_tensor("kvbv", [CKV, HG * D_V], BF16, kind="ExternalInput")
    ow = nc.dram_tensor("ow", [HG * D_V, HID], BF16, kind="ExternalInput")
    cos_s = nc.dram_tensor("cos_s", [S_SH, D_ROPE], F32, kind="ExternalInput")
    sin_sg = nc.dram_tensor("sin_sg", [S_SH, D_ROPE], F32, kind="ExternalInput")
    cos2t = nc.dram_tensor("cos2t", [128, S], BF16, kind="ExternalInput")
    sin2tg = nc.dram_tensor("sin2tg", [128, S], BF16, kind="ExternalInput")
    masks = nc.dram_tensor("masks", [512, 512], BF16, kind="ExternalInput")
    ones_col = nc.dram_tensor("ones_col", [128, 1], BF16, kind="ExternalInput")
    ones_row = nc.dram_tensor("ones_row", [1, 128], BF16, kind="ExternalInput")
    out = nc.dram_tensor("out", [S, HID], F32, kind="ExternalOutput")

    with tile.TileContext(nc) as tc, ExitStack() as top:
        const = top.enter_context(tc.tile_pool(name="const", bufs=1))
        outsp = top.enter_context(tc.tile_pool(name="outsp", bufs=1))
        dram = top.enter_context(tc.tile_pool(name="dram", bufs=1, space="DRAM"))
        # ---- constants in SBUF ----
        ident = const.tile([128, 128], BF16, tag="ident")
        make_identity(nc, ident[:])
        masks_sb = const.tile([128, 4, 512], BF16, tag="masks")
        for m in range(4):
            nc.sync.dma_start(masks_sb[:, m, :], masks.ap()[m * 128:(m + 1) * 128, :])
        cos_s_sb = const.tile([S_SH, D_ROPE], F32, tag="coss")
        sin_sg_sb = const.tile([S_SH, D_ROPE], F32, tag="sinsg")
        nc.sync.dma_start(cos_s_sb[:], cos_s.ap())
        nc.sync.dma_start(sin_sg_sb[:], sin_sg.ap())
        cos2t_sb = const.tile([128, S], BF16, tag="cos2t")
        sin2tg_sb = const.tile([128, S], BF16, tag="sin2tg")
        nc.sync.dma_start(cos2t_sb[:], cos2t.ap())
        nc.sync.dma_start(sin2tg_sb[:], sin2tg.ap())
        ones_col_sb = const.tile([128, 1], BF16, tag="onesc")
        ones_row_sb = const.tile([1, 128], BF16, tag="onesr")
        nc.sync.dma_start(ones_col_sb[:], ones_col.ap())
        nc.sync.dma_start(ones_row_sb[:], ones_row.ap())

        # attention outputs for all 16 local heads, [dv, head, s], bf16
        outs_all = outsp.tile([128, HG, S], BF16, tag="outs_all")

        CKP = CKV + D_ROPE  # 576 ckv+kpe rows
        CQH = CQ // 2
        agi_q1 = dram.tile([CQH, S_SH], BF16, tag="agi_q1")
        ago_q1 = dram.tile([CQH * N_CORES, S_SH], BF16, tag="ago_q1")
        agi_q2 = dram.tile([CQH, S_SH], BF16, tag="agi_q2")
        ago_q2 = dram.tile([CQH * N_CORES, S_SH], BF16, tag="ago_q2")
        agi_kv = dram.tile([CKP, S_SH], BF16, tag="agi_kv")
        ago_kv = dram.tile([CKP * N_CORES, S_SH], BF16, tag="ago_kv")

        # ================= Phase A: fused a-proj + rmsnorm + kpe rope ======
        with ExitStack() as pa:
            sba = pa.enter_context(tc.tile_pool(name="sba", bufs=1))
            sbw = pa.enter_context(tc.tile_pool(name="sbw", bufs=3))
            sbt = pa.enter_context(tc.tile_pool(name="sbt", bufs=2))
            ps_a = pa.enter_context(tc.tile_pool(name="ps_a", bufs=1, space="PSUM"))
            ps_tp = pa.enter_context(tc.tile_pool(name="ps_tp", bufs=2, space="PSUM"))

            xT_sb = sba.tile([128, CC_A, S_SH], BF16, tag="xT")
            nc.sync.dma_start(xT_sb[:, 0, :], xT.ap()[0:128, :])
            # 5 live psum banks accumulate the full [128, 2112] activation row
            pa_t = [ps_a.tile([128, 512], F32, name=f"pa{nt}", tag=f"pa{nt}")
                    for nt in range(5)]
            for cc in range(CC_A):
                wt = sbw.tile([128, CA], BF16, tag="wa")
                nc.sync.dma_start(wt[:], wa.ap()[cc * 128:(cc + 1) * 128, :])
                if cc + 1 < CC_A:
                    nc.sync.dma_start(xT_sb[:, cc + 1, :],
                                      xT.ap()[(cc + 1) * 128:(cc + 2) * 128, :])
                for nt, (d0, dn) in enumerate(NT_A):
                    nc.tensor.matmul(pa_t[nt][:, :dn], xT_sb[:, cc, :],
                                     wt[:, d0:d0 + dn],
                                     start=(cc == 0), stop=(cc == CC_A - 1))
            acts = sba.tile([S_SH, CA], F32, tag="acts")
            for nt, (d0, dn) in enumerate(NT_A):
                nc.scalar.copy(acts[:, d0:d0 + dn], pa_t[nt][:, :dn])

            # rmsnorm factors for qc (cols 0:1536) and ckv (cols 1536:2048)
            sq = sba.tile([S_SH, CQ + CKV], F32, tag="sq")
            nc.vector.tensor_mul(sq[:], acts[:, 0:CQ + CKV], acts[:, 0:CQ + CKV])
            fq = sbt.tile([S_SH, 1], F32, tag="fq")
            fk = sbt.tile([S_SH, 1], F32, tag="fk")
            nc.vector.reduce_sum(fq[:], sq[:, 0:CQ], axis=mybir.AxisListType.X)
            nc.vector.reduce_sum(fk[:], sq[:, CQ:CQ + CKV], axis=mybir.AxisListType.X)
            nc.vector.tensor_scalar(fq[:], fq[:], 1.0 / CQ, EPS, OP.mult, OP.add)
            nc.vector.tensor_scalar(fk[:], fk[:], 1.0 / CKV, EPS, OP.mult, OP.add)
            nc.vector.reciprocal(fq[:], fq[:])
            nc.vector.reciprocal(fk[:], fk[:])
            nc.scalar.activation(fq[:], fq[:], AF.Sqrt)
            nc.scalar.activation(fk[:], fk[:], AF.Sqrt)
            acts_bf = sba.tile([S_SH, CA], BF16, tag="acts_bf")
            nc.vector.tensor_scalar_mul(acts_bf[:, 0:CQ], acts[:, 0:CQ], fq[:])
            nc.vector.tensor_scalar_mul(acts_bf[:, CQ:CQ + CKV],
                                        acts[:, CQ:CQ + CKV], fk[:])

            # k_pe rope (natural [s, 64] layout), cols 2048:2112
            kp0 = CQ + CKV
            kv1 = sbt.tile([S_SH, D_ROPE], F32, tag="kv1")
            kv2 = sbt.tile([S_SH, D_ROPE], F32, tag="kv2")
            nc.vector.tensor_mul(kv1[:], acts[:, kp0:kp0 + 64], cos_s_sb[:])
            nc.vector.tensor_mul(kv2[:, 0:32], acts[:, kp0 + 32:kp0 + 64], sin_sg_sb[:, 0:32])
            nc.vector.tensor_mul(kv2[:, 32:64], acts[:, kp0:kp0 + 32], sin_sg_sb[:, 32:64])
            nc.vector.tensor_add(acts_bf[:, kp0:kp0 + 64], kv1[:], kv2[:])

            # transpose all 17 chunks -> bounce buffers (kv chunks first so
            # the kv AllGather can start early)
            bT = sba.tile([128, 17 * 128], BF16, tag="bT")
            for t in list(range(12, 17)) + list(range(12)):
                w = 128 if t < 16 else 64
                pt = ps_tp.tile([128, 512], BF16, tag="projbf")
                nc.tensor.transpose(pt[:w, 0:128], acts_bf[:, t * 128:t * 128 + w],
                                    ident[:])
                nc.scalar.copy(bT[:w, t * 128:(t + 1) * 128], pt[:w, 0:128])
                if t < 6:
                    nc.sync.dma_start(agi_q1[t * 128:t * 128 + w, :],
                                      bT[:w, t * 128:(t + 1) * 128])
                elif t < 12:
                    r0 = (t - 6) * 128
                    nc.sync.dma_start(agi_q2[r0:r0 + w, :],
                                      bT[:w, t * 128:(t + 1) * 128])
                else:
                    r0 = (t - 12) * 128
                    nc.sync.dma_start(agi_kv[r0:r0 + w, :],
                                      bT[:w, t * 128:(t + 1) * 128])

        ps_proj = top.enter_context(tc.tile_pool(name="ps_proj", bufs=3, space="PSUM"))

        nc.gpsimd.collective_compute(
            "AllGather", OP.bypass,
            replica_groups=[list(range(N_CORES))],
            ins=[agi_kv.opt()], outs=[ago_kv.opt()],
        )
        nc.gpsimd.collective_compute(
            "AllGather", OP.bypass,
            replica_groups=[list(range(N_CORES))],
            ins=[agi_q1.opt()], outs=[ago_q1.opt()],
        )
        nc.gpsimd.collective_compute(
            "AllGather", OP.bypass,
            replica_groups=[list(range(N_CORES))],
            ins=[agi_q2.opt()], outs=[ago_q2.opt()],
        )

        # ================= Phase B: per-head-group projections + attention ==
        with ExitStack() as pb:
            ps_sc = pb.enter_context(tc.tile_pool(name="ps_sc", bufs=3, space="PSUM"))
            ps_ao = pb.enter_context(tc.tile_pool(name="ps_ao", bufs=2, space="PSUM"))
            sbg = pb.enter_context(tc.tile_pool(name="sbg", bufs=1))
            sbwq = pb.enter_context(tc.tile_pool(name="sbwq", bufs=2))
            sbh = pb.enter_context(tc.tile_pool(name="sbh", bufs=2))
            sbp = pb.enter_context(tc.tile_pool(name="sbp", bufs=2))
            sbv = pb.enter_context(tc.tile_pool(name="sbv", bufs=2))
            sbs = pb.enter_context(tc.tile_pool(name="sbs", bufs=3))

            def load_group_weights(g):
                h0 = g * G_HEADS
                qbnw = sbwq.tile([128, CQ // 128, G_HEADS * 128], BF16,
                                 name=f"qbnw{g}", tag="qbnw")
                qbpw = sbwq.tile([128, CQ // 128, G_HEADS * 64], BF16,
                                 name=f"qbpw{g}", tag="qbpw")
                kvbkw = sbwq.tile([128, CKV // 128, G_HEADS * 128], BF16,
                                  name=f"kvbkw{g}", tag="kvbkw")
                kvbvw = sbwq.tile([128, CKV // 128, G_HEADS * 128], BF16,
                                  name=f"kvbvw{g}", tag="kvbvw")
                for c in range(CQ // 128):
                    nc.sync.dma_start(qbnw[:, c, :], qbn.ap()[c * 128:(c + 1) * 128,
                                                              h0 * 128:(h0 + G_HEADS) * 128])
                    nc.sync.dma_start(qbpw[:, c, :], qbp.ap()[c * 128:(c + 1) * 128,
                                                              h0 * 64:(h0 + G_HEADS) * 64])
                for c in range(CKV // 128):
                    nc.sync.dma_start(kvbkw[:, c, :], kvbk.ap()[c * 128:(c + 1) * 128,
                                                                h0 * 128:(h0 + G_HEADS) * 128])
                    nc.sync.dma_start(kvbvw[:, c, :], kvbv.ap()[c * 128:(c + 1) * 128,
                                                                h0 * 128:(h0 + G_HEADS) * 128])
                return qbnw, qbpw, kvbkw, kvbvw

            wtiles = {0: load_group_weights(0)}

            # gathered activations, stitched per 512-wide s-tile
            qct = []
            ckv = []
            for st in range(2):
                q_t = sbg.tile([128, CQ // 128, 512], BF16, tag=f"qct{st}")
                k_t = sbg.tile([128, CKV // 128, 512], BF16, tag=f"ckv{st}")
                for r in range(4):
                    core = st * 4 + r
                    for c in range(CKV // 128):
                        nc.sync.dma_start(
                            k_t[:, c, r * 128:(r + 1) * 128],
                            ago_kv[core * CKP + c * 128:core * CKP + (c + 1) * 128, :])
                    for c in range(CQ // 128):
                        if c < 6:
                            nc.sync.dma_start(
                                q_t[:, c, r * 128:(r + 1) * 128],
                                ago_q1[core * CQH + c * 128:
                                       core * CQH + (c + 1) * 128, :])
                        else:
                            c2 = c - 6
                            nc.sync.dma_start(
                                q_t[:, c, r * 128:(r + 1) * 128],
                                ago_q2[core * CQH + c2 * 128:
                                       core * CQH + (c2 + 1) * 128, :])
                qct.append(q_t)
                ckv.append(k_t)
            srows = sbg.tile([HG * QT, 512], F32, tag="srows")
            rec_all = sbg.tile([HG * QT, 512], BF16, tag="rec_all")
            kpe2 = sbg.tile([128, S], BF16, tag="kpe2")
            for core in range(N_CORES):
                base = core * CKP + CKV
                nc.sync.dma_start(kpe2[0:64, core * 128:(core + 1) * 128],
                                  ago_kv[base:base + 64, :])
                nc.sync.dma_start(kpe2[64:128, core * 128:(core + 1) * 128],
                                  ago_kv[base:base + 64, :])

            for g in range(N_GROUPS):
                h0 = g * G_HEADS
                if g + 1 < N_GROUPS:
                    wtiles[g + 1] = load_group_weights(g + 1)
                qbnw, qbpw, kvbkw, kvbvw = wtiles.pop(g)

                # --- kv-side first (only needs the kv AllGather) ---
                kTn = []
                for i in range(G_HEADS):
                    kt_t = sbh.tile([128, S], BF16, tag="kTn")
                    ps2 = [ps_proj.tile([128, 512], F32, name=f"ps2_{st}", tag="proj")
                           for st in range(2)]
                    for c in range(CKV // 128):
                        for st in range(2):
                            nc.tensor.matmul(ps2[st][:],
                                             kvbkw[:, c, i * 128:(i + 1) * 128],
                                             ckv[st][:, c, :],
                                             start=(c == 0), stop=(c == CKV // 128 - 1))
                    for st in range(2):
                        nc.scalar.copy(kt_t[:, st * 512:(st + 1) * 512], ps2[st][:])
                    kTn.append(kt_t)

                v_g = sbv.tile([128, 8, G_HEADS * 128], BF16, tag="v_g")
                for sc in range(8):
                    st = sc // 4
                    psum = ps_proj.tile([128, 512], F32, tag="proj")
                    nn = G_HEADS * 128
                    for c in range(CKV // 128):
                        nc.tensor.matmul(
                            psum[:, :nn],
                            ckv[st][:, c, (sc % 4) * 128:(sc % 4 + 1) * 128],
                            kvbvw[:, c, :],
                            start=(c == 0), stop=(c == CKV // 128 - 1))
                    nc.vector.tensor_copy(v_g[:, sc, :], psum[:, :nn])

                # --- q-side (needs the q AllGather) ---
                qTn = []
                for i in range(G_HEADS):
                    qt_t = sbh.tile([128, S], BF16, tag="qTn")
                    ps2 = [ps_proj.tile([128, 512], F32, name=f"ps2_{st}", tag="proj")
                           for st in range(2)]
                    for c in range(CQ // 128):
                        for st in range(2):
                            nc.tensor.matmul(ps2[st][:],
                                             qbnw[:, c, i * 128:(i + 1) * 128],
                                             qct[st][:, c, :],
                                             start=(c == 0), stop=(c == CQ // 128 - 1))
                    for st in range(2):
                        nc.scalar.copy(qt_t[:, st * 512:(st + 1) * 512], ps2[st][:])
                    qTn.append(qt_t)

                qp_raw = sbp.tile([128, S], BF16, tag="qp_raw")
                ps2 = [ps_proj.tile([128, 512], F32, name=f"ps2_{st}", tag="proj")
                       for st in range(2)]
                for c in range(CQ // 128):
                    for st in range(2):
                        nc.tensor.matmul(ps2[st][:], qbpw[:, c, :], qct[st][:, c, :],
                                         start=(c == 0), stop=(c == CQ // 128 - 1))
                for st in range(2):
                    nc.scalar.copy(qp_raw[:, st * 512:(st + 1) * 512], ps2[st][:])
                # rope on the head-pair tile: rows [0:64]=head h0, [64:128]=h0+1
                qTp = sbh.tile([128, S], BF16, tag="qTp")
                rm = sbp.tile([128, S], BF16, tag="ropem")
                rs = sbp.tile([128, S], BF16, tag="ropes")
                nc.vector.tensor_mul(rm[:], qp_raw[:], cos2t_sb[:])
                # rs = swap32(qp_raw), then multiply by the sign-baked sin table
                for b in range(4):
                    r0 = b * 32
                    r1 = r0 + 32 if b % 2 == 0 else r0 - 32
                    nc.vector.tensor_copy(rs[r0:r0 + 32, :], qp_raw[r1:r1 + 32, :])
                nc.vector.tensor_mul(rs[:], rs[:], sin2tg_sb[:])
                nc.vector.tensor_add(qTp[:], rm[:], rs[:])

                # --- attention for each head in the group ---
                for i in range(G_HEADS):
                    h_loc = h0 + i
                    for qt in range(QT):
                        kmax = 4 * (qt + 1)
                        psum_o = ps_ao.tile([128, 512], F32, tag="o")
                        sums = sbs.tile([128, 512], F32, tag="sums")
                        for kc in range(kmax):
                            ps = ps_sc.tile([128, 512], F32, tag="s")
                            nc.tensor.matmul(ps[:], kTn[i][:, kc * 128:(kc + 1) * 128],
                                             qTn[i][:, qt * 512:(qt + 1) * 512],
                                             start=True, stop=False)
                            b = i * 64
                            nc.tensor.matmul(ps[:], kpe2[b:b + 64, kc * 128:(kc + 1) * 128],
                                             qTp[b:b + 64, qt * 512:(qt + 1) * 512],
                                             start=False, stop=True)
                            pt = sbs.tile([128, 512], BF16, tag="pt")
                            nc.scalar.activation(pt[:], ps[:], AF.Exp, scale=SCALE)
                            if kc >= 4 * qt:
                                nc.vector.tensor_mul(pt[:], pt[:],
                                                     masks_sb[:, kc - 4 * qt, :])
                            if kc == 0:
                                nc.vector.tensor_copy(sums[:], pt[:])
                            else:
                                nc.vector.tensor_add(sums[:], sums[:], pt[:])
                            nc.tensor.matmul(psum_o[:],
                                             v_g[:, kc, i * 128:(i + 1) * 128], pt[:],
                                             start=(kc == 0), stop=(kc == kmax - 1))
                        sums_bf = sbs.tile([128, 512], BF16, tag="sums_bf")
                        nc.vector.tensor_copy(sums_bf[:], sums[:])
                        pss = ps_sc.tile([128, 512], F32, tag="s")
                        nc.tensor.matmul(pss[0:1, :], ones_col_sb[:], sums_bf[:],
                                         start=True, stop=True)
                        row = h_loc * QT + qt
                        srow = sbs.tile([1, 512], F32, tag="srow")
                        nc.scalar.copy(srow[:], pss[0:1, :])
                        nc.sync.dma_start(srows[row:row + 1, :], srow[:])
                        nc.scalar.copy(outs_all[:, h_loc, qt * 512:(qt + 1) * 512],
                                       psum_o[:])

            # --- deferred softmax normalization: one batched reciprocal ---
            with nc.allow_low_precision(reason="softmax recip in bf16"):
                nc.vector.reciprocal(rec_all[:], srows[:])
            for h_loc in range(HG):
                for qt in range(QT):
                    row = h_loc * QT + qt
                    rrow = sbs.tile([1, 512], BF16, tag="rrow")
                    nc.sync.dma_start(rrow[:], rec_all[row:row + 1, :])
                    psb = ps_sc.tile([128, 512], F32, tag="s")
                    nc.tensor.matmul(psb[:], ones_row_sb[:], rrow[:],
                                     start=True, stop=True)
                    bsb = sbs.tile([128, 512], BF16, tag="bsb")
                    nc.scalar.copy(bsb[:], psb[:])
                    nc.vector.tensor_mul(
                        outs_all[:, h_loc, qt * 512:(qt + 1) * 512],
                        outs_all[:, h_loc, qt * 512:(qt + 1) * 512], bsb[:])

        # ================= Phase C: partial output projection ===============
        with ExitStack() as pc:
            sbow = pc.enter_context(tc.tile_pool(name="sbow", bufs=2))
            sbos = pc.enter_context(tc.tile_pool(name="sbos", bufs=3))
            for nt in range(HID // 512):
                owt = sbow.tile([128, HG, 512], BF16, tag="ow")
                for hc in range(HG):
                    nc.sync.dma_start(owt[:, hc, :],
                                      ow.ap()[hc * 128:(hc + 1) * 128,
                                              nt * 512:(nt + 1) * 512])
                for st in range(8):
                    psum = ps_proj.tile([128, 512], F32, tag="proj")
                    for hc in range(HG):
                        nc.tensor.matmul(psum[:],
                                         outs_all[:, hc, st * 128:(st + 1) * 128],
                                         owt[:, hc, :],
                                         start=(hc == 0), stop=(hc == HG - 1))
                    osb = sbos.tile([128, 512], F32, tag="osb")
                    nc.scalar.copy(osb[:], psum[:])
                    nc.sync.dma_start(out.ap()[st * 128:(st + 1) * 128,
                                               nt * 512:(nt + 1) * 512], osb[:])

    nc.compile()
    return nc


def _host_inputs(hidden_states, position_ids, q_a_weight, q_a_layernorm_weight,
                 q_b_weight, kv_a_weight, kv_a_layernorm_weight, kv_b_weight,
                 o_weight):
    bf = ml_dtypes.bfloat16
    x = np.asarray(hidden_states, np.float32).reshape(S, HID)
    pos = np.asarray(position_ids, np.float64).reshape(S)
    q_a_w = np.asarray(q_a_weight, np.float32)
    q_ln = np.asarray(q_a_layernorm_weight, np.float32)
    q_b_w = np.asarray(q_b_weight, np.float32)
    kv_a_w = np.asarray(kv_a_weight, np.float32)
    kv_ln = np.asarray(kv_a_layernorm_weight, np.float32)
    kv_b_w = np.asarray(kv_b_weight, np.float32)
    o_w = np.asarray(o_weight, np.float32)

    wa = np.concatenate([q_a_w, kv_a_w], axis=1).astype(bf)    # [HID, 2112]
    xT = np.ascontiguousarray(x.T).astype(bf)                   # [HID, S]

    # fold the rms-norm weights into the b-projections
    qb = (q_ln[:, None] * q_b_w).reshape(CQ, H, D_Q)
    kvb = (kv_ln[:, None] * kv_b_w).reshape(CKV, H, D_NOPE + D_V)

    # rope tables
    inv_freq = 1.0 / (10000.0 ** (np.arange(0, D_ROPE, 2, dtype=np.float64) / D_ROPE))
    freqs = pos[:, None] * inv_freq[None, :]                # [S, 32]
    emb = np.concatenate([freqs, freqs], axis=-1)           # [S, 64]
    cos = np.cos(emb).astype(np.float32)
    sin = np.sin(emb).astype(np.float32)
    sin_sg = np.concatenate([-sin[:, :32], sin[:, 32:]], axis=1)  # [S, 64]
    cosT = np.ascontiguousarray(cos.T)                      # [64, S]
    sinT_sg = np.ascontiguousarray(sin_sg.T)                # [64, S]
    cos2t = np.concatenate([cosT, cosT], axis=0).astype(bf)     # [128, S]
    sin2tg = np.concatenate([sinT_sg, sinT_sg], axis=0).astype(bf)  # [128, S]

    # causal masks for the 4 diagonal offsets
    masks = np.zeros((4, 128, 512), np.float32)
    i = np.arange(128)[:, None]
    j = np.arange(512)[None, :]
    for m in range(4):
        masks[m] = ((i + m * 128) <= j).astype(np.float32)
    masks = masks.reshape(512, 512).astype(bf)

    ones_col = np.ones((128, 1), np.float32).astype(bf)
    ones_row = np.ones((1, 128), np.float32).astype(bf)

    in_maps = []
    for c in range(N_CORES):
        hs = slice(c * HG, (c + 1) * HG)
        in_maps.append({
            "xT": np.ascontiguousarray(xT[:, c * S_SH:(c + 1) * S_SH]),
            "wa": wa,
            "qbn": np.ascontiguousarray(
                qb[:, hs, :D_NOPE].reshape(CQ, HG * D_NOPE)).astype(bf),
            "qbp": np.ascontiguousarray(
                qb[:, hs, D_NOPE:].reshape(CQ, HG * D_ROPE)).astype(bf),
            "kvbk": np.ascontiguousarray(
                kvb[:, hs, :D_NOPE].reshape(CKV, HG * D_NOPE)).astype(bf),
            "kvbv": np.ascontiguousarray(
                kvb[:, hs, D_NOPE:].reshape(CKV, HG * D_V)).astype(bf),
            "ow": np.ascontiguousarray(
                o_w[c * HG * D_V:(c + 1) * HG * D_V, :]).astype(bf),
            "cos_s": np.ascontiguousarray(cos[c * S_SH:(c + 1) * S_SH, :]),
            "sin_sg": np.ascontiguousarray(sin_sg[c * S_SH:(c + 1) * S_SH, :]),
            "cos2t": cos2t,
            "sin2tg": sin2tg,
            "masks": masks,
            "ones_col": ones_col,
            "ones_row": ones_row,
        })
    return in_maps


def kernel(**inputs):
    global LAST_EXEC_NS
    trace = bool(inputs.pop("_trace", False))
    in_maps = _host_inputs(**inputs)
    if "nc" not in _CACHE:
        _CACHE["nc"] = _build_nc()
    nc = _CACHE["nc"]
    res = bass_utils.run_bass_kernel_spmd(
        nc, in_maps, core_ids=list(range(N_CORES)), trace=trace)
    LAST_EXEC_NS = res.exec_time_ns
    total = np.zeros((S, HID), np.float64)
    for c in range(N_CORES):
        total += res.results[c]["out"].astype(np.float64)
    return total.astype(np.float32).reshape(1, 1, S, HID)
